# revision 1
# baseline (speedup 1.0000x reference)
"""Trainium2 Bass kernel for nn_Attention_34840774705279 (sparse/deformable attention).

Math (matches reference.py):
  v   = x @ v_w.T + v_b            -> per-head maps [B*NH, H, W, HD]
  off = x @ off_w.T + off_b        -> off_w is structurally zero, so offsets are
                                      CONSTANT per (head, point); for this problem
                                      they are (+-p or ~1e-16) => integer shifts.
  w   = softmax_p(x @ aw_w.T + aw_b)
  out[i,j] = sum_p w_p[i,j] * v[i+dy_p, j+dx_p]   (zero outside the map)
  y   = out @ proj_w.T + proj_b

Sharding (8 cores, uniform SPMD program):
  core d -> batch b = d//2, row-half r0 = 64*(d%2). Each core computes ALL 8
  heads for its 64 output rows (8192 tokens) using a 4-row halo of v rows
  (host zero-pads x rows outside the image), so shifts up to +-4 never cross
  cores and no cross-core reduction is needed; the host just concatenates.

Device algorithm (per core):
  A. v+logit projection, pixel-major: host supplies x TRANSPOSED [256, 9216];
     per image row r: a[j, 0:288] = xT_row_r.T @ [v_w.T | aw_w.T] lands
     [j=col, (9 slots x 32)] in one PSUM bank and is copied once into the
     merged VL tile [j, 9, 72, 32] (slots 0-7 = per-head v maps, slot 8 =
     attention logits). All matmuls run in float32r (full PE rate at N>=256).
  B. softmax over the 4 points, batched across all 8 heads (exp on ScalarE,
     adds/reciprocal on VectorE), split by row-half into separate E tiles.
  C. sampling + weighting via weight-then-shift identity
        w .* (S_dx @ V_win) == S_dx @ ((S_-dx^T w) .* V_win):
     per (head, point): one tiny matmul computes the column-shifted weights
     E' = S_-dx^T E (alpha folded in), VectorE multiplies the 32-row V window
     (row shift dy = compile-time slot offset) by E' broadcast over d, and the
     0/1 column-shift matrix S_dx matmul-accumulates all 4 points directly in
     PSUM. Bilinear (fractional) offsets are supported as multiple corner
     terms; integer offsets (this problem) are a single term each.
  D. output projection: PE transposes OUT rows back to channel-major,
     y^T = proj.T @ OUT^T accumulated over both 128-channel halves; host
     transposes y^T back. proj_b applied via ScalarE bias (zero-bias inputs
     skip the bias matmuls entirely).
  Emission interleaves C-half0 into phase A's tail and D-half0 into C-half1
  so VectorE weighting overlaps PE/DMA work of neighbouring phases.
"""

import os
import sys
import math

import numpy as np

sys.path.insert(0, "/opt/trn_rl_repo")

P = 128
H = W = 128
NH, NP, HD = 8, 4, 32
DIM = 256
N_TOK = H * W
ROWS_OUT = 64          # output rows per core
HALO = 4
ROWS_V = ROWS_OUT + 2 * HALO   # 72 v-row slots per core
TOK_V = ROWS_V * W             # 9216
N_CORES = 8

F32R = os.environ.get("KERNEL_F32R", "1") == "1"

_cache = {}


def _build_terms(off_b):
    """Per (h, p): list of (dx, dy, alpha) corner terms from the constant offsets.

    General for any constant offset (bilinear corners); for this problem each
    (h, p) yields exactly one term with alpha ~= 1."""
    ob = np.asarray(off_b, np.float64).reshape(NH, NP, 2)
    terms = [[[] for _ in range(NP)] for _ in range(NH)]
    for h in range(NH):
        for p in range(NP):
            fx, fy = ob[h, p, 0], ob[h, p, 1]
            x0 = math.floor(fx)
            y0 = math.floor(fy)
            wx1 = fx - x0
            wy1 = fy - y0
            for dxc, wx in ((x0, 1.0 - wx1), (x0 + 1, wx1)):
                if abs(wx) < 1e-9:
                    continue
                for dyc, wy in ((y0, 1.0 - wy1), (y0 + 1, wy1)):
                    if abs(wy) < 1e-9:
                        continue
                    if abs(dxc) >= W or abs(dyc) > HALO:
                        continue  # fully out of range / beyond halo
                    terms[h][p].append((int(dxc), int(dyc), float(wx * wy)))
    return terms


def _build_smats(terms):
    """Dedupe (dx, alpha) -> [128,128] shift matrices; rewrite terms to
    (s_fwd, s_bwd, dy): out += S_dx @ (V_window * (alpha*S_-dx^T E))."""
    key_to_idx = {}
    mats = []

    def smat(dx, alpha):
        key = (dx, round(alpha, 9))
        if key not in key_to_idx:
            m = np.zeros((P, P), np.float32)
            for j_out in range(W):
                j_in = j_out + dx
                if 0 <= j_in < W:
                    m[j_in, j_out] = alpha
            key_to_idx[key] = len(mats)
            mats.append(m)
        return key_to_idx[key]

    terms2 = [[[] for _ in range(NP)] for _ in range(NH)]
    for h in range(NH):
        for p in range(NP):
            for dx, dy, alpha in terms[h][p]:
                terms2[h][p].append(
                    (smat(dx, 1.0), smat(-dx, alpha), dy))
    return np.stack(mats, 0), terms2


def _np_reference(x, v_w, v_b, aw_w, aw_b, off_w, off_b, proj_w, proj_b, Hh, Ww):
    """Pure-numpy fallback mirroring reference.py (used only if off_w != 0,
    which cannot happen with this problem's setup_inputs)."""
    B, N, C = x.shape
    v = (x @ v_w.T + v_b).reshape(B, N, NH, HD).transpose(0, 2, 1, 3)
    v = v.reshape(B * NH, Hh, Ww, HD)
    mh, mw = np.meshgrid(np.arange(Hh, dtype=x.dtype), np.arange(Ww, dtype=x.dtype),
                         indexing="ij")
    ref = np.stack([mw, mh], -1).reshape(1, N, 1, 2)
    off = (x @ off_w.T + off_b).reshape(B, N, NH, NP, 2).transpose(0, 2, 1, 3, 4)
    off = off.reshape(B * NH, N, NP, 2)
    grid = ref + off
    w = (x @ aw_w.T + aw_b).reshape(B, N, NH, NP).transpose(0, 2, 1, 3)
    w = w.reshape(B * NH, N, NP)
    w = np.exp(w - w.max(-1, keepdims=True))
    w = w / w.sum(-1, keepdims=True)
    G = B * NH
    vf = v.reshape(G, Hh * Ww, HD)
    gx, gy = grid[..., 0], grid[..., 1]
    x0 = np.floor(gx); y0 = np.floor(gy)
    wx1 = gx - x0; wx0 = 1.0 - wx1
    wy1 = gy - y0; wy0 = 1.0 - wy1
    x0i = x0.astype(np.int64); y0i = y0.astype(np.int64)

    def gather(xi, yi):
        valid = (xi >= 0) & (xi < Ww) & (yi >= 0) & (yi < Hh)
        idx = (np.clip(yi, 0, Hh - 1) * Ww + np.clip(xi, 0, Ww - 1))
        g = np.take_along_axis(vf, idx.reshape(G, -1, 1), axis=1)
        return g.reshape(*xi.shape, HD) * valid[..., None]

    samp = ((wy0 * wx0)[..., None] * gather(x0i, y0i)
            + (wy0 * wx1)[..., None] * gather(x0i + 1, y0i)
            + (wy1 * wx0)[..., None] * gather(x0i, y0i + 1)
            + (wy1 * wx1)[..., None] * gather(x0i + 1, y0i + 1))
    out = np.einsum("gnpd,gnp->gnd", samp, w)
    out = out.reshape(B, NH, N, HD).transpose(0, 2, 1, 3).reshape(B, N, C)
    return (out @ proj_w.T + proj_b).astype(np.float32)


def _build_program(terms, n_smats, has_bias=True):
    import concourse.bass as bass
    import concourse.mybir as mybir
    import concourse.tile as tile
    from concourse import bacc

    dt = mybir.dt
    f32 = dt.float32

    fr = dt.float32r if F32R else f32

    nc = bacc.Bacc("TRN2", target_bir_lowering=False, debug=False,
                   num_devices=N_CORES)

    NCH = 256 + NH * NP  # 288: v channels + aw logits per row matmul

    # ---- DRAM I/O ----
    xt_d = nc.dram_tensor("xt_dev", [DIM, TOK_V], fr, kind="ExternalInput")
    ones_d = nc.dram_tensor("ones_dev", [1, TOK_V], fr, kind="ExternalInput")
    wb_d = nc.dram_tensor("wb_cat", [2, P, NCH], fr, kind="ExternalInput")
    bb_d = nc.dram_tensor("bb_cat", [1, NCH], fr, kind="ExternalInput")
    s_d = nc.dram_tensor("s_mats", [n_smats, P, P], fr, kind="ExternalInput")
    pj_d = nc.dram_tensor("proj_t", [2, 2, P, P], fr, kind="ExternalInput")
    pb_d = nc.dram_tensor("projb_t", [2, P], f32, kind="ExternalInput")
    id_d = nc.dram_tensor("ident", [P, P], fr, kind="ExternalInput")
    y0_d = nc.dram_tensor("y0", [P, ROWS_OUT * W], f32, kind="ExternalOutput")
    y1_d = nc.dram_tensor("y1", [P, ROWS_OUT * W], f32, kind="ExternalOutput")
    y_outs = [y0_d, y1_d]

    NG = ROWS_V // 4           # 18 x-DMA groups of 4 rows
    NGO = ROWS_OUT // 4        # 16 groups for phase D

    with tile.TileContext(nc) as tc:
        with (
            tc.tile_pool(name="const", bufs=1) as cpool,
            tc.tile_pool(name="big", bufs=1) as bigpool,
        ):
            # ---- constants ----
            wb_sb = cpool.tile([P, 2, NCH], fr, tag="wb")
            nc.sync.dma_start(wb_sb[:], wb_d.rearrange("kc k f -> k kc f"))
            bb_sb = cpool.tile([1, NCH], fr, tag="bb")
            nc.sync.dma_start(bb_sb[:], bb_d[:])
            s_sb = cpool.tile([P, n_smats, P], fr, tag="smats")
            nc.sync.dma_start(s_sb[:], s_d.rearrange("s k f -> k s f"))
            pj_sb = cpool.tile([P, 2, 2, P], fr, tag="proj")
            nc.sync.dma_start(pj_sb[:], pj_d.rearrange("kc m k f -> k kc m f"))
            pb_sb = cpool.tile([P, 2], f32, tag="projb")
            nc.sync.dma_start(pb_sb[:], pb_d.rearrange("m k -> k m"))
            id_sb = cpool.tile([P, P], fr, tag="ident")
            nc.sync.dma_start(id_sb[:], id_d[:])

            # ---- persistent big tiles ----
            vl_sb = bigpool.tile([P, NH + 1, ROWS_V, HD], fr, tag="V")
            v_sb = vl_sb[:, :NH]
            outs = [bigpool.tile([P, 32, 2, P], fr, tag="OUT", name="out0"),
                    bigpool.tile([P, 32, 2, P], fr, tag="OUT2", name="out1")]
            es = [bigpool.tile([P, NH * NP, 32], fr, tag="E", name="e0"),
                  bigpool.tile([P, NH * NP, 32], fr, tag="E2", name="e1")]

            abc_pools = (
                tc.tile_pool(name="stA", bufs=2),
                tc.tile_pool(name="psA", bufs=2, space="PSUM"),
                tc.tile_pool(name="psC", bufs=2, space="PSUM"),
                tc.tile_pool(name="wt", bufs=1),
                tc.tile_pool(name="stB", bufs=2),
            )
            stA = abc_pools[0].__enter__()
            psA = abc_pools[1].__enter__()
            psC = abc_pools[2].__enter__()
            wtpool = abc_pools[3].__enter__()
            stB = abc_pools[4].__enter__()

            def phase_a(g):
                """x rows 4g..4g+4: v-proj + logits, pixel-major."""
                tok0 = g * 512
                xt_g = [stA.tile([P, 512], fr, tag=f"xt{kc}", bufs=3,
                                 name=f"xtg{kc}") for kc in range(2)]
                for kc in range(2):
                    nc.sync.dma_start(
                        xt_g[kc][:],
                        xt_d[P * kc:P * kc + P, tok0:tok0 + 512])
                if has_bias:
                    ones_g = stA.tile([1, 512], fr, tag="ones")
                    nc.sync.dma_start(ones_g[:], ones_d[:, tok0:tok0 + 512])
                for rl in range(4):
                    rr = 4 * g + rl      # v-row slot
                    a_ps = psA.tile([P, 512], f32, tag="a_ps", bufs=4)
                    for kc in range(2):
                        nc.tensor.matmul(
                            a_ps[:, :NCH],
                            xt_g[kc][:, P * rl:P * rl + P],
                            wb_sb[:, kc, :], start=(kc == 0),
                            stop=(kc == 1 and not has_bias))
                    if has_bias:
                        nc.tensor.matmul(
                            a_ps[:, :NCH], ones_g[:, P * rl:P * rl + P],
                            bb_sb[:], start=False, stop=True)
                    nc.scalar.copy(
                        vl_sb[:, :, rr, :],
                        a_ps[:, :NCH].rearrange("j (h d) -> j h d", h=NH + 1))

            def phase_b(half, heads=None):
                """exp + softmax over points, all heads, rows of `half`."""
                rr = 32 * half
                e_sb = es[half]
                nc.scalar.activation(
                    e_sb[:].rearrange("j hp i -> j i hp"),
                    vl_sb[:, NH, HALO + rr:HALO + rr + 32, :]
                    .rearrange("j i d -> j i d"),
                    mybir.ActivationFunctionType.Exp)
                z = stB.tile([P, NH, 32], f32, tag="z")
                zr = stB.tile([P, NH, 32], fr, tag="zr")
                ev = e_sb[:].rearrange("j (h p) i -> j h p i", p=NP)
                nc.vector.tensor_tensor(z[:], ev[:, :, 0, :], ev[:, :, 1, :],
                                        op=mybir.AluOpType.add)
                nc.vector.tensor_tensor(z[:], z[:], ev[:, :, 2, :],
                                        op=mybir.AluOpType.add)
                nc.vector.tensor_tensor(z[:], z[:], ev[:, :, 3, :],
                                        op=mybir.AluOpType.add)
                with nc.allow_low_precision(reason="fp32r == fp32 bits"):
                    nc.vector.reciprocal(zr[:], z[:])
                for p in range(NP):
                    nc.vector.tensor_tensor(ev[:, :, p, :], ev[:, :, p, :],
                                            zr[:], op=mybir.AluOpType.mult)

            def phase_c(half, heads=None):
                """weight-then-shift: out += S_dx @ (V_win * (S_-dx^T w))."""
                rr = 32 * half
                e_sb = es[half]
                for h in (range(NH) if heads is None else heads):
                    mh, hl = h // 4, h % 4
                    o_ps = psC.tile([P, 32, HD], f32, tag="oacc", bufs=1)
                    n_terms = sum(len(terms[h][p]) for p in range(NP))
                    t_seen = 0
                    for p in range(NP):
                        for (s_fwd, s_bwd, dy) in terms[h][p]:
                            ep_ps = psC.tile([P, 32], f32, tag="ep", bufs=2)
                            nc.tensor.matmul(
                                ep_ps[:], s_sb[:, s_bwd, :],
                                e_sb[:, 4 * h + p, :], start=True, stop=True)
                            ep = wtpool.tile([P, 32], fr, tag="ep_sb",
                                             bufs=2, name="ep")
                            nc.scalar.copy(ep[:], ep_ps[:])
                            m_t = wtpool.tile([P, 32, HD], fr,
                                              tag=f"wt{t_seen % 2}",
                                              bufs=2, name=f"mt{t_seen % 2}")
                            slot0 = rr + dy + HALO
                            nc.vector.tensor_tensor(
                                m_t[:], v_sb[:, h, slot0:slot0 + 32, :],
                                ep[:].unsqueeze(2).broadcast_to([P, 32, HD]),
                                op=mybir.AluOpType.mult)
                            for ch in range(2):
                                nc.tensor.matmul(
                                    o_ps[:, 16 * ch:16 * ch + 16, :]
                                    .rearrange("j i d -> j (i d)"),
                                    s_sb[:, s_fwd, :],
                                    m_t[:, 16 * ch:16 * ch + 16, :]
                                    .rearrange("j i d -> j (i d)"),
                                    start=(t_seen == 0),
                                    stop=(t_seen == n_terms - 1))
                            t_seen += 1
                    nc.scalar.copy(
                        outs[half][:, :, mh, HD * hl:HD * hl + HD],
                        o_ps[:])

            # ---- emission order: A(<40 rows), B0, C0 overlap A(rest), B1, C1
            for g in range(10):
                phase_a(g)
            phase_b(0)
            ctail = list(range(10, NG))
            def phase_d(halfd, gls=None, evac_dve=False):
                """output projection for row groups of half `halfd`."""
                for gl in (range(NGO // 2) if gls is None else gls):
                    g = halfd * (NGO // 2) + gl
                    i0 = 4 * g
                    ot_sb = []
                    for m in range(2):
                        ot_ps = psA.tile([P, 4, P], fr, tag="a_ps",
                                         name=f"ot{m}", bufs=4)
                        for c in range(4):
                            nc.tensor.transpose(
                                ot_ps[:, c, :],
                                outs[halfd][:, i0 - 32 * halfd + c, m, :],
                                id_sb[:])
                        t = stA.tile([P, 512], fr, tag=f"ot{m}", bufs=1)
                        (nc.vector.tensor_copy if evac_dve else nc.scalar.copy)(
                            t[:], ot_ps[:].rearrange("k c f -> k (c f)"))
                        ot_sb.append(t)
                    for mc in range(2):
                        y_ps = psA.tile([P, 512], f32, tag="a_ps",
                                        name=f"yps{mc}", bufs=4)
                        for kc in range(2):
                            nc.tensor.matmul(y_ps[:], pj_sb[:, kc, mc, :],
                                             ot_sb[kc][:],
                                             start=(kc == 0), stop=(kc == 1))
                        ysb = stA.tile([P, 512], f32, tag=f"y{mc}",
                                       name=f"ysb{mc}")
                        if evac_dve:
                            nc.vector.scalar_tensor_tensor(
                                ysb[:], y_ps[:], 1.0,
                                pb_sb[:, mc:mc + 1].to_broadcast([P, 512]),
                                op0=mybir.AluOpType.mult,
                                op1=mybir.AluOpType.add)
                        else:
                            nc.scalar.activation(
                                ysb[:], y_ps[:],
                                mybir.ActivationFunctionType.Identity,
                                bias=pb_sb[:, mc:mc + 1])
                        nc.sync.dma_start(
                            y_outs[mc][:, 512 * g:512 * g + 512], ysb[:])

            PH = do_c = os.environ.get("KERNEL_PHASES", "abcd")
            nc0 = NH if "c" in PH else (1 if "x" in PH else 0)
            ci = 0
            for i, g in enumerate(ctail):
                phase_a(g)
                if ci < nc0:
                    phase_c(0, heads=[ci]); ci += 1
            for h in range(ci, nc0):
                phase_c(0, heads=[h])
            if "c" in PH:
                phase_b(1)
            for h in range(NH):
                if "c" in PH:
                    phase_c(1, heads=[h])
                if "d" in PH:
                    phase_d(0, gls=[h])
            if "d" in PH:
                phase_d(1)
            for pl in reversed(abc_pools):
                pl.__exit__(None, None, None)

    nc.compile()
    return nc

def kernel(x, v_w, v_b, aw_w, aw_b, off_w, off_b, proj_w, proj_b, H=128, W=128,
           **_unused):
    x = np.ascontiguousarray(np.asarray(x, np.float32))
    v_w = np.asarray(v_w, np.float32); v_b = np.asarray(v_b, np.float32)
    aw_w = np.asarray(aw_w, np.float32); aw_b = np.asarray(aw_b, np.float32)
    off_w = np.asarray(off_w, np.float32); off_b = np.asarray(off_b, np.float32)
    proj_w = np.asarray(proj_w, np.float32); proj_b = np.asarray(proj_b, np.float32)

    if np.any(off_w != 0.0) or int(H) != 128 or int(W) != 128:
        # data-dependent offsets or non-128 map: exact host fallback
        return _np_reference(x, v_w, v_b, aw_w, aw_b, off_w, off_b,
                             proj_w, proj_b, int(H), int(W))

    terms = _build_terms(off_b)
    s_mats, terms2 = _build_smats(terms)

    has_bias = bool(np.any(v_b) or np.any(aw_b))
    key = ("prog", s_mats.shape[0], has_bias,
           tuple(tuple(tuple(tl) for tl in th) for th in terms2))
    if key not in _cache:
        _cache[key] = _build_program(terms2, s_mats.shape[0], has_bias)
    nc = _cache[key]

    B = x.shape[0]
    # ---- host prep, shared across cores ----
    NCH = 256 + NH * NP
    wb_cat = np.empty((2, P, NCH), np.float32)
    for kc in range(2):
        wb_cat[kc, :, :256] = v_w[:, P * kc:P * (kc + 1)].T
        wb_cat[kc, :, 256:] = aw_w[:, P * kc:P * (kc + 1)].T
    bb_cat = np.concatenate([v_b, aw_b]).reshape(1, NCH)
    pj_t = np.empty((2, 2, P, P), np.float32)
    for kc in range(2):
        for mc in range(2):
            pj_t[kc, mc] = proj_w[P * mc:P * (mc + 1), P * kc:P * (kc + 1)].T
    pb_t = proj_b.reshape(2, P)
    ident = np.eye(P, dtype=np.float32)
    shared = dict(wb_cat=np.ascontiguousarray(wb_cat),
                  bb_cat=np.ascontiguousarray(bb_cat),
                  s_mats=np.ascontiguousarray(s_mats),
                  proj_t=np.ascontiguousarray(pj_t),
                  projb_t=np.ascontiguousarray(pb_t),
                  ident=ident)

    xr = x.reshape(B, H, W, DIM)
    in_maps = []
    for d in range(N_CORES):
        b, half = d // 2, d % 2
        r0 = ROWS_OUT * half
        x_dev = np.zeros((ROWS_V, W, DIM), np.float32)
        ones = np.zeros((ROWS_V, W), np.float32)
        lo, hi = max(0, r0 - HALO), min(H, r0 + ROWS_OUT + HALO)
        x_dev[lo - (r0 - HALO):hi - (r0 - HALO)] = xr[b, lo:hi]
        ones[lo - (r0 - HALO):hi - (r0 - HALO)] = 1.0
        m = dict(shared)
        m["xt_dev"] = np.ascontiguousarray(x_dev.reshape(TOK_V, DIM).T)
        m["ones_dev"] = ones.reshape(1, TOK_V)
        in_maps.append(m)

    from concourse import bass_utils
    res = bass_utils.run_bass_kernel_spmd(
        nc, in_maps, core_ids=list(range(N_CORES)),
        trace=os.environ.get("KERNEL_TRACE", "0") == "1")
    kernel.last_results = res

    y = np.empty((B, N_TOK, DIM), np.float32)
    for d in range(N_CORES):
        b, half = d // 2, d % 2
        yd = np.concatenate([res.results[d]["y0"], res.results[d]["y1"]], 0)
        y[b, ROWS_OUT * W * half:ROWS_OUT * W * (half + 1), :] = yd.T
    return y



# revision 30
# speedup vs baseline: 1.5238x; 1.5238x over previous
"""Trainium2 Bass kernel for nn_Attention_34840774705279 (sparse/deformable attention).

Math (matches reference.py):
  v   = x @ v_w.T                  -> per-head maps [B*NH, H, W, HD]
  off = off_b (off_w == 0)         -> constant integer offsets (dx, dy) = p*(ux, uy)
  w   = softmax_p(x @ aw_w.T)
  out[i,j] = sum_p w_p[i,j] * v[i+dy_p, j+dx_p]   (zero outside the map)
  y   = out @ proj_w.T

Sharding (8 cores, uniform SPMD): core d -> batch b = d//2, row-half r0 =
64*(d%2); each core computes all 8 heads for its 64 output rows using a 4-row
halo of v rows (host zero-pads), so no cross-core traffic; host concatenates.

v2 design (all bf16 data path, f32 PSUM accumulation):
  A. pixel-major projection: per image row r, a_ps[j, 288] = x_row^T @ [v|aw]
     (2 matmuls, contraction 256); pairs of rows share one PSUM tile and one
     ScalarE evacuation copy into VL[j, 288ch, 72slots] (slot innermost so the
     DVE weighting op later hits its 2-byte fast path).
  B. softmax over points: one exp (ScalarE), adds + reciprocal (DVE, f32),
     one batched normalize multiply (DVE bf16 2x mode).
  C. weight-then-shift: heads are host-permuted into [+dx trio, -dx trio,
     dx=0 duo] so the column-shifted weights E' = S_-dx^T E are computed with
     ONE matmul per dx value (3 heads batched); DVE multiplies the V window by
     E' (bf16 2x); 0/1 shift matrices S_dx matmul-accumulate the 4 points in
     PSUM (the p-sum rides the PSUM accumulation for free).
  D. output projection: PE transposes OUT rows to channel-major (bf16 PSUM),
     y^T = proj^T @ OUT^T; y leaves as bf16, host casts + transposes.
  Evac copies are spread over ScalarE/Pool(GpSimd)/DVE to balance engines.
"""

import os
import sys
import math

import numpy as np

sys.path.insert(0, "/opt/trn_rl_repo")

P = 128
H = W = 128
NH, NP, HD = 8, 4, 32
DIM = 256
N_TOK = H * W
ROWS_OUT = 64          # output rows per core
HALO = 4
ROWS_V = ROWS_OUT + 2 * HALO   # 72 v-row slots per core
TOK_V = ROWS_V * W             # 9216
N_CORES = 8
NCH = DIM + NH * NP    # 288 channels out of the fused projection

_cache = {}


# ---------------------------------------------------------------------------
# geometry: constant offsets -> head permutation + shift matrices
# ---------------------------------------------------------------------------

def _derive_geometry(off_b):
    """For each head h expect offsets (dx, dy) = p*(ux, uy), ux/uy in {-1,0,1},
    integer (bilinear weight ~1). Returns (hord, uys, uxs) with heads permuted
    to [ux=+1 trio, ux=-1 trio, ux=0 duo], or None if the pattern fails."""
    ob = np.asarray(off_b, np.float64).reshape(NH, NP, 2)
    info = []
    for h in range(NH):
        u = None
        for p in range(NP):
            fx, fy = ob[h, p, 0], ob[h, p, 1]
            dx, dy = round(fx), round(fy)
            # must be integer shifts with negligible bilinear remainder
            if abs(fx - dx) > 1e-6 or abs(fy - dy) > 1e-6:
                return None
            if dx % (p + 1) or dy % (p + 1):
                return None
            uu = (dx // (p + 1), dy // (p + 1))
            if abs(uu[0]) > 1 or abs(uu[1]) > 1:
                return None
            if u is None:
                u = uu
            elif u != uu:
                return None
        info.append(u)
    plus = [h for h in range(NH) if info[h][0] == 1]
    minus = [h for h in range(NH) if info[h][0] == -1]
    zero = [h for h in range(NH) if info[h][0] == 0]
    if len(plus) != 3 or len(minus) != 3 or len(zero) != 2:
        return None
    hord = plus + minus + zero
    uys = [info[h][1] for h in hord]
    uxs = [info[h][0] for h in hord]
    return hord, uys, uxs


def _build_smats():
    """smat[k] for k in -4..4: m[j_in, j_out] = 1 at j_in = j_out + k."""
    mats = np.zeros((9, P, P), np.float32)
    for k in range(-4, 5):
        m = mats[k + 4]
        for j_out in range(W):
            j_in = j_out + k
            if 0 <= j_in < W:
                m[j_in, j_out] = 1.0
    return mats


def _np_reference(x, v_w, v_b, aw_w, aw_b, off_w, off_b, proj_w, proj_b, Hh, Ww):
    """Pure-numpy fallback mirroring reference.py (only used off-spec)."""
    B, N, C = x.shape
    v = (x @ v_w.T + v_b).reshape(B, N, NH, HD).transpose(0, 2, 1, 3)
    v = v.reshape(B * NH, Hh, Ww, HD)
    mh, mw = np.meshgrid(np.arange(Hh, dtype=x.dtype), np.arange(Ww, dtype=x.dtype),
                         indexing="ij")
    ref = np.stack([mw, mh], -1).reshape(1, N, 1, 2)
    off = (x @ off_w.T + off_b).reshape(B, N, NH, NP, 2).transpose(0, 2, 1, 3, 4)
    off = off.reshape(B * NH, N, NP, 2)
    grid = ref + off
    w = (x @ aw_w.T + aw_b).reshape(B, N, NH, NP).transpose(0, 2, 1, 3)
    w = w.reshape(B * NH, N, NP)
    w = np.exp(w - w.max(-1, keepdims=True))
    w = w / w.sum(-1, keepdims=True)
    G = B * NH
    vf = v.reshape(G, Hh * Ww, HD)
    gx, gy = grid[..., 0], grid[..., 1]
    x0 = np.floor(gx); y0 = np.floor(gy)
    wx1 = gx - x0; wx0 = 1.0 - wx1
    wy1 = gy - y0; wy0 = 1.0 - wy1
    x0i = x0.astype(np.int64); y0i = y0.astype(np.int64)

    def gather(xi, yi):
        valid = (xi >= 0) & (xi < Ww) & (yi >= 0) & (yi < Hh)
        idx = (np.clip(yi, 0, Hh - 1) * Ww + np.clip(xi, 0, Ww - 1))
        g = np.take_along_axis(vf, idx.reshape(G, -1, 1), axis=1)
        return g.reshape(*xi.shape, HD) * valid[..., None]

    samp = ((wy0 * wx0)[..., None] * gather(x0i, y0i)
            + (wy0 * wx1)[..., None] * gather(x0i + 1, y0i)
            + (wy1 * wx0)[..., None] * gather(x0i, y0i + 1)
            + (wy1 * wx1)[..., None] * gather(x0i + 1, y0i + 1))
    out = np.einsum("gnpd,gnp->gnd", samp, w)
    out = out.reshape(B, NH, N, HD).transpose(0, 2, 1, 3).reshape(B, N, C)
    return (out @ proj_w.T + proj_b).astype(np.float32)


# ---------------------------------------------------------------------------
# device program
# ---------------------------------------------------------------------------

def _build_program(uys):
    import concourse.bass as bass
    import concourse.mybir as mybir
    import concourse.tile as tile
    from concourse import bacc

    dt = mybir.dt
    f32 = dt.float32
    bf16 = dt.bfloat16

    # engine placement knobs. PSUM evacuations may only use scalar/vector
    # (GPSIMD cannot access PSUM on HW); pool takes SBUF-only weighting ops.
    EV_A = os.environ.get("EV_A", "sv")        # rotation for A evacs
    EV_OUT = os.environ.get("EV_OUT", "s")     # rotation for out evacs
    EV_OT0 = os.environ.get("EV_OT0", "s")     # D0 ot evacs
    EV_OT1 = os.environ.get("EV_OT1", "v")     # D1 ot evacs
    EV_Y0 = os.environ.get("EV_Y0", "sv")      # D0 y evacs (mc0, mc1)
    EV_Y1 = os.environ.get("EV_Y1", "ss")      # D1 y evacs
    WEIGHT_ROT = os.environ.get("WEIGHT_ROT", "vvvp")  # weighting ops rotation
    ENG = {"s": "scalar", "v": "vector", "p": "pool"}

    nc = bacc.Bacc("TRN2", target_bir_lowering=False, debug=False,
                   num_devices=N_CORES)

    _rotc = {}

    def evac(which):
        if len(which) > 1:  # rotation string like "sv"
            i = _rotc[which] = (_rotc.get(which, -1) + 1) % len(which)
            which = which[i]
        return {"s": nc.scalar.copy, "v": nc.vector.tensor_copy,
                "scalar": nc.scalar.copy, "vector": nc.vector.tensor_copy,
                "pool": nc.gpsimd.tensor_copy}[which]

    def tt_eng(rot):
        i = _rotc[rot] = (_rotc.get(rot, -1) + 1) % len(rot)
        return {"v": nc.vector, "p": nc.gpsimd, "s": None}[rot[i]]

    # ---- DRAM I/O ----
    xt_d = nc.dram_tensor("xt_dev", [DIM, TOK_V], bf16, kind="ExternalInput")
    wb_d = nc.dram_tensor("wb_cat", [2, P, NCH], bf16, kind="ExternalInput")
    s_d = nc.dram_tensor("s_mats", [9, P, P], bf16, kind="ExternalInput")
    pj_d = nc.dram_tensor("proj_t", [2, 2, P, P], bf16, kind="ExternalInput")
    id_d = nc.dram_tensor("ident", [P, P], bf16, kind="ExternalInput")
    y_d = [nc.dram_tensor(f"y{mc}", [P, ROWS_OUT * W], bf16,
                          kind="ExternalOutput") for mc in range(2)]

    NG = 9  # x DMA groups of 8 rows

    with tile.TileContext(nc) as tc:
        with (
            tc.tile_pool(name="const", bufs=1) as cpool,
            tc.tile_pool(name="big", bufs=1) as bigpool,
            tc.tile_pool(name="stA", bufs=2) as stA,
            tc.tile_pool(name="stB", bufs=2) as stB,
            tc.tile_pool(name="stM", bufs=1) as stM,
            tc.tile_pool(name="stD", bufs=2) as stD,
            tc.tile_pool(name="psA", bufs=2, space="PSUM") as psA,
            tc.tile_pool(name="psC", bufs=2, space="PSUM") as psC,
        ):
            # ---- constants (only wb gates phase A; rest loaded later) ----
            wb_sb = cpool.tile([P, 2, NCH], bf16, tag="wb")
            nc.sync.dma_start(wb_sb[:], wb_d.rearrange("kc k f -> k kc f"))
            s_sb = cpool.tile([P, 9, P], bf16, tag="smats")
            pj_sb = cpool.tile([P, 2, 2, P], bf16, tag="proj")
            id_sb = cpool.tile([P, P], bf16, tag="ident")

            def load_late_consts():
                nc.sync.dma_start(s_sb[:], s_d.rearrange("s k f -> k s f"))
                nc.sync.dma_start(pj_sb[:],
                                  pj_d.rearrange("kc m k f -> k kc m f"))
                nc.sync.dma_start(id_sb[:], id_d[:])

            def sf(dx):  # forward shift matrix index
                return dx + 4

            def sb(dx):  # backward (S_-dx) index
                return -dx + 4

            # ---- persistent tiles ----
            # VL: [j, 288 ch, 72 row-slots]; ch = h'*32+d for v, 256+h'*4+p logits
            vl = bigpool.tile([P, NCH, ROWS_V], bf16, tag="VL")
            es = [bigpool.tile([P, NH * NP, 32], bf16, tag="E", name=f"e{h}")
                  for h in range(2)]
            eps = [bigpool.tile([P, 8, 3, 32], bf16, tag="EP", name=f"ep{h}")
                   for h in range(2)]
            outs = [bigpool.tile([P, DIM, 32], bf16, tag="OUT", name=f"out{h}")
                    for h in range(2)]

            def phase_a(row0, nrows, early):
                """x rows row0..row0+nrows: fused v+logit projection.

                Single-row PSUM tiles with a 4-deep ring: the evac round-trip
                (copy + 2 sem hops ~= 850ns) hides under 4 rows of PE work."""
                xt_g = [stA.tile([P, 2048], bf16, tag=f"xt{kc}", bufs=3,
                                 name=f"xtg{kc}") for kc in range(2)]
                for kc in range(2):
                    nc.sync.dma_start(
                        xt_g[kc][:, :P * nrows],
                        xt_d[P * kc:P * kc + P,
                             P * row0:P * (row0 + nrows)])
                for rl in range(nrows):
                    a_ps = psA.tile([P, 512], f32, tag="a", bufs=4)
                    for kc in range(2):
                        nc.tensor.matmul(
                            a_ps[:, :NCH],
                            xt_g[kc][:, P * rl:P * rl + P],
                            wb_sb[:, kc, :], start=(kc == 0),
                            stop=(kc == 1))
                    s0 = row0 + rl
                    evac(EV_A)(vl[:, :, s0], a_ps[:, :NCH])

            def phase_b(half):
                """exp + softmax over the 4 points, all heads, 32 rows."""
                rr = 32 * half
                e_sb = es[half]
                nc.scalar.activation(
                    e_sb[:], vl[:, DIM:, HALO + rr:HALO + rr + 32],
                    mybir.ActivationFunctionType.Exp)
                ev = e_sb[:].rearrange("j (h p) i -> j h p i", p=NP)
                z0 = stB.tile([P, NH, 32], f32, tag="z0")
                z1 = stB.tile([P, NH, 32], f32, tag="z1")
                zr = stB.tile([P, NH, 32], f32, tag="zr")
                zb = stB.tile([P, NH, 32], bf16, tag="zb")
                nc.vector.tensor_tensor(z0[:], ev[:, :, 0, :], ev[:, :, 1, :],
                                        op=mybir.AluOpType.add)
                nc.vector.tensor_tensor(z1[:], ev[:, :, 2, :], ev[:, :, 3, :],
                                        op=mybir.AluOpType.add)
                nc.vector.tensor_tensor(z0[:], z0[:], z1[:],
                                        op=mybir.AluOpType.add)
                nc.vector.reciprocal(zr[:], z0[:])
                nc.vector.tensor_copy(zb[:], zr[:])
                nc.vector.tensor_tensor(
                    ev[:], ev[:],
                    zb[:].unsqueeze(2).broadcast_to([P, NH, NP, 32]),
                    op=mybir.AluOpType.mult)

            def phase_ep(half):
                """column-shifted weights E' = S_-dx^T E, batched 3 heads/dx."""
                e_sb = es[half]
                for rnd in range(2):           # rnd 0: dx=+1..+4, 1: dx=-1..-4
                    sgn = 1 if rnd == 0 else -1
                    hoff = 0 if rnd == 0 else 3
                    ep_ps = psC.tile([P, 4, 128], f32, tag="ep", bufs=1)
                    for k in range(1, 5):
                        dx = sgn * k
                        # heads hoff..hoff+2, point k-1: slots h'*4+(k-1)
                        rhs = e_sb[:].rearrange("j (h p) i -> j h p i", p=NP)[
                            :, hoff:hoff + 3, k - 1, :]
                        nc.tensor.matmul(ep_ps[:, k - 1, :96],
                                         s_sb[:, sb(dx), :], rhs,
                                         start=True, stop=True)
                    nc.scalar.copy(
                        eps[half][:, 4 * rnd:4 * rnd + 4],
                        ep_ps[:, :, :96].rearrange("j k (t i) -> j k t i", t=3))

            def _weight_slice(half, hp, p):
                """weight row [P, 32] for (permuted head hp, point p)."""
                if hp < 6:
                    rnd = 0 if hp < 3 else 1
                    return eps[half][:, 4 * rnd + p, hp % 3, :]
                return es[half][:, NP * hp + p, :]

            def phase_c_heads(half, heads, mtag, out_eng=None):
                """weighting + shift-accumulate + evac for a group of heads."""
                rr = 32 * half
                m_t = stM.tile([P, 3, NP, 32, 32], bf16,
                               tag=mtag, bufs=1, name=mtag)
                for t, hp in enumerate(heads):
                    uy = uys[hp]
                    for p in range(NP):
                        s0 = rr + HALO + uy * (p + 1)
                        tt_eng(WEIGHT_ROT).tensor_tensor(
                            m_t[:, t, p],
                            vl[:, HD * hp:HD * hp + HD, s0:s0 + 32],
                            _weight_slice(half, hp, p)
                            .unsqueeze(1).broadcast_to([P, HD, 32]),
                            op=mybir.AluOpType.mult)
                if heads[0] >= 6 and os.environ.get("DUO_DVE", "0") == "1":
                    # dx = 0 duo: point-sum on DVE (no shift, no PSUM, no evac)
                    for t, hp in enumerate(heads):
                        t0 = stM.tile([P, 32, 32], bf16, tag="duo0", bufs=2)
                        t1 = stM.tile([P, 32, 32], bf16, tag="duo1", bufs=2)
                        nc.vector.tensor_tensor(t0[:], m_t[:, t, 0],
                                                m_t[:, t, 1],
                                                op=mybir.AluOpType.add)
                        nc.vector.tensor_tensor(t1[:], m_t[:, t, 2],
                                                m_t[:, t, 3],
                                                op=mybir.AluOpType.add)
                        nc.vector.tensor_tensor(
                            outs[half][:, HD * hp:HD * hp + HD, :],
                            t0[:], t1[:], op=mybir.AluOpType.add)
                    return
                for t, hp in enumerate(heads):
                    dxu = 1 if hp < 3 else (-1 if hp < 6 else 0)
                    o_ps = [psC.tile([P, 512], f32, tag="o", bufs=3,
                                     name=f"o{ch}") for ch in range(2)]
                    for p in range(NP):
                        dx = dxu * (p + 1)
                        mv = m_t[:, t, p].rearrange("j d i -> j (d i)")
                        for ch in range(2):
                            nc.tensor.matmul(
                                o_ps[ch][:], s_sb[:, sf(dx), :],
                                mv[:, 512 * ch:512 * ch + 512],
                                start=(p == 0), stop=(p == NP - 1))
                    for ch in range(2):
                        evac(out_eng or EV_OUT)(
                            outs[half][:, HD * hp + 16 * ch:HD * hp + 16 * ch + 16, :],
                            o_ps[ch][:].rearrange("j (d i) -> j d i", d=16))

            # ---- phase D: software-pipelined (trans(g+1) emitted before
            # proj(g) so PE never stalls on the ot evacuation copy) ----
            d_state = {}

            def d_trans(half, gl, ot_eng=None):
                ot_eng = ot_eng or (EV_OT0 if half == 0 else EV_OT1)
                """transposes of out-row group gl to channel-major."""
                i0 = 4 * gl
                ot_ps = psA.tile([P, 8, P], bf16, tag="a",
                                 name="otp", bufs=4)
                for kc in range(2):
                    for il in range(4):
                        nc.tensor.transpose(
                            ot_ps[:, 4 * kc + il, :],
                            outs[half][:, P * kc:P * kc + P, i0 + il],
                            id_sb[:])
                ot_sb = stD.tile([P, 2, 512], bf16, tag="ot", bufs=2)
                evac(ot_eng)(
                    ot_sb[:].rearrange("j kc f -> j (kc f)"),
                    ot_ps[:].rearrange("j a b -> j (a b)"))
                d_state[(half, gl)] = ot_sb

            def d_proj(half, gl, y_engs=None):
                """output projection + y evac (+DMA every 4th group)."""
                y_engs = y_engs or (EV_Y0 if half == 0 else EV_Y1)
                g = 8 * half + gl
                ot_sb = d_state.pop((half, gl))
                y_ps = [psA.tile([P, 512], f32, tag="a",
                                 name=f"yps{mc}", bufs=4) for mc in range(2)]
                for mc in range(2):
                    for kc in range(2):
                        nc.tensor.matmul(
                            y_ps[mc][:], pj_sb[:, kc, mc, :],
                            ot_sb[:, kc, :],
                            start=(kc == 0), stop=(kc == 1))
                q = gl % 2
                ysb = d_state.get(("ysb", half, gl // 2))
                if ysb is None:
                    ysb = stD.tile([P, 2, 2, 512], bf16, tag="y", bufs=3)
                    d_state[("ysb", half, gl // 2)] = ysb
                for mc in range(2):
                    evac(y_engs[mc])(ysb[:, mc, q, :], y_ps[mc][:])
                if q == 1:
                    g0 = 512 * (g - 1)
                    for mc in range(2):
                        nc.sync.dma_start(
                            y_d[mc][:, g0:g0 + 1024], ysb[:, mc, :, :])

            # ---- emission ----
            phase_a(0, 8, True)
            phase_a(8, 8, True)
            load_late_consts()
            phase_a(16, 16, True)
            phase_a(32, 16, True)
            phase_b(0)           # no PE work; overlaps A tail
            phase_a(48, 16, False)
            phase_ep(0)
            phase_c_heads(0, [0, 1, 2], "m0")
            phase_a(64, 8, False)
            phase_c_heads(0, [3, 4, 5], "m1")
            phase_c_heads(0, [6, 7], "m0")
            phase_b(1)
            d_trans(0, 0)   # fill the B1 latency with D0 groups
            d_trans(0, 1); d_proj(0, 0)
            d_trans(0, 2); d_proj(0, 1)
            phase_ep(1)
            d_trans(0, 3); d_proj(0, 2)
            phase_c_heads(1, [0, 1, 2], "m1")
            d_trans(0, 4); d_proj(0, 3)
            d_trans(0, 5); d_proj(0, 4)
            phase_c_heads(1, [3, 4, 5], "m0")
            d_trans(0, 6); d_proj(0, 5)
            d_trans(0, 7); d_proj(0, 6)
            phase_c_heads(1, [6, 7], "m1")
            d_proj(0, 7)
            d_trans(1, 0)
            for gl in range(1, 8):
                d_trans(1, gl); d_proj(1, gl - 1)
            d_proj(1, 7)

    nc.compile()
    return nc


# ---------------------------------------------------------------------------
# host wrapper
# ---------------------------------------------------------------------------

def kernel(x, v_w, v_b, aw_w, aw_b, off_w, off_b, proj_w, proj_b, H=128, W=128,
           **_unused):
    import ml_dtypes
    bf = ml_dtypes.bfloat16

    x = np.ascontiguousarray(np.asarray(x, np.float32))
    v_w = np.asarray(v_w, np.float32); v_b = np.asarray(v_b, np.float32)
    aw_w = np.asarray(aw_w, np.float32); aw_b = np.asarray(aw_b, np.float32)
    off_w = np.asarray(off_w, np.float32); off_b = np.asarray(off_b, np.float32)
    proj_w = np.asarray(proj_w, np.float32); proj_b = np.asarray(proj_b, np.float32)

    geom = _derive_geometry(off_b)
    if (np.any(off_w != 0.0) or int(H) != 128 or int(W) != 128 or geom is None
            or np.any(v_b) or np.any(aw_b) or np.any(proj_b)):
        return _np_reference(x, v_w, v_b, aw_w, aw_b, off_w, off_b,
                             proj_w, proj_b, int(H), int(W))
    hord, uys, _uxs = geom

    key = ("prog2", tuple(uys))
    if key not in _cache:
        _cache[key] = _build_program(uys)
    nc = _cache[key]

    B = x.shape[0]
    # ---- host prep (shared across cores) ----
    # channel permutation: ch' = h'*32+d  <-  orig h*32+d
    vperm = np.concatenate([np.arange(HD) + HD * h for h in hord])
    aperm = np.concatenate([np.arange(NP) + NP * h for h in hord])
    wb_cat = np.empty((2, P, NCH), np.float32)
    v_wp = v_w[vperm]          # [256 out-ch', 256 in]
    aw_wp = aw_w[aperm]        # [32 out-ch', 256 in]
    for kc in range(2):
        wb_cat[kc, :, :DIM] = v_wp[:, P * kc:P * (kc + 1)].T
        wb_cat[kc, :, DIM:] = aw_wp[:, P * kc:P * (kc + 1)].T
    pj_perm = proj_w[:, vperm]  # permute contraction columns
    pj_t = np.empty((2, 2, P, P), np.float32)
    for kc in range(2):
        for mc in range(2):
            pj_t[kc, mc] = pj_perm[P * mc:P * (mc + 1), P * kc:P * (kc + 1)].T
    shared = dict(wb_cat=np.ascontiguousarray(wb_cat).astype(bf),
                  s_mats=np.ascontiguousarray(_build_smats()).astype(bf),
                  proj_t=np.ascontiguousarray(pj_t).astype(bf),
                  ident=np.eye(P, dtype=np.float32).astype(bf))

    xr = x.reshape(B, H, W, DIM)
    in_maps = []
    for d in range(N_CORES):
        b, half = d // 2, d % 2
        r0 = ROWS_OUT * half
        x_dev = np.zeros((ROWS_V, W, DIM), np.float32)
        lo, hi = max(0, r0 - HALO), min(H, r0 + ROWS_OUT + HALO)
        x_dev[lo - (r0 - HALO):hi - (r0 - HALO)] = xr[b, lo:hi]
        m = dict(shared)
        m["xt_dev"] = np.ascontiguousarray(
            x_dev.reshape(TOK_V, DIM).T).astype(bf)
        in_maps.append(m)

    from concourse import bass_utils
    res = bass_utils.run_bass_kernel_spmd(
        nc, in_maps, core_ids=list(range(N_CORES)),
        trace=os.environ.get("KERNEL_TRACE", "0") == "1")
    kernel.last_results = res

    y = np.empty((B, N_TOK, DIM), np.float32)
    for d in range(N_CORES):
        b, half = d // 2, d % 2
        yd = np.concatenate([np.asarray(res.results[d]["y0"]),
                             np.asarray(res.results[d]["y1"])], 0)
        y[b, ROWS_OUT * W * half:ROWS_OUT * W * (half + 1), :] = \
            yd.astype(np.float32).T
    return y


# revision 51
# speedup vs baseline: 1.5593x; 1.0233x over previous
"""Trainium2 Bass kernel for nn_Attention_34840774705279 (sparse/deformable attention).

Math (matches reference.py):
  v   = x @ v_w.T                  -> per-head maps [B*NH, H, W, HD]
  off = off_b (off_w == 0)         -> constant integer offsets (dx, dy) = p*(ux, uy)
  w   = softmax_p(x @ aw_w.T)
  out[i,j] = sum_p w_p[i,j] * v[i+dy_p, j+dx_p]   (zero outside the map)
  y   = out @ proj_w.T

Sharding (8 cores, uniform SPMD): core d -> batch b = d//2, row-half r0 =
64*(d%2); each core computes all 8 heads for its 64 output rows using a 4-row
halo of v rows (host zero-pads), so no cross-core traffic; host concatenates.

v2 design (all bf16 data path, f32 PSUM accumulation):
  A. pixel-major projection: per image row r, a_ps[j, 288] = x_row^T @ [v|aw]
     (2 matmuls, contraction 256); pairs of rows share one PSUM tile and one
     ScalarE evacuation copy into VL[j, 288ch, 72slots] (slot innermost so the
     DVE weighting op later hits its 2-byte fast path).
  B. softmax over points: one exp (ScalarE), adds + reciprocal (DVE, f32),
     one batched normalize multiply (DVE bf16 2x mode).
  C. weight-then-shift: heads are host-permuted into [+dx trio, -dx trio,
     dx=0 duo] so the column-shifted weights E' = S_-dx^T E are computed with
     ONE matmul per dx value (3 heads batched); DVE multiplies the V window by
     E' (bf16 2x); 0/1 shift matrices S_dx matmul-accumulate the 4 points in
     PSUM (the p-sum rides the PSUM accumulation for free).
  D. output projection: PE transposes OUT rows to channel-major (bf16 PSUM),
     y^T = proj^T @ OUT^T; y leaves as bf16, host casts + transposes.
  Evac copies are spread over ScalarE/Pool(GpSimd)/DVE to balance engines.
"""

import os
import sys
import math

import numpy as np

sys.path.insert(0, "/opt/trn_rl_repo")

P = 128
H = W = 128
NH, NP, HD = 8, 4, 32
DIM = 256
N_TOK = H * W
ROWS_OUT = 64          # output rows per core
HALO = 4
ROWS_V = ROWS_OUT + 2 * HALO   # 72 v-row slots per core
TOK_V = ROWS_V * W             # 9216
N_CORES = 8
NCH = DIM + NH * NP    # 288 channels out of the fused projection

_cache = {}


# ---------------------------------------------------------------------------
# geometry: constant offsets -> head permutation + shift matrices
# ---------------------------------------------------------------------------

def _derive_geometry(off_b):
    """For each head h expect offsets (dx, dy) = p*(ux, uy), ux/uy in {-1,0,1},
    integer (bilinear weight ~1). Returns (hord, uys, uxs) with heads permuted
    to [ux=+1 trio, ux=-1 trio, ux=0 duo], or None if the pattern fails."""
    ob = np.asarray(off_b, np.float64).reshape(NH, NP, 2)
    info = []
    for h in range(NH):
        u = None
        for p in range(NP):
            fx, fy = ob[h, p, 0], ob[h, p, 1]
            dx, dy = round(fx), round(fy)
            # must be integer shifts with negligible bilinear remainder
            if abs(fx - dx) > 1e-6 or abs(fy - dy) > 1e-6:
                return None
            if dx % (p + 1) or dy % (p + 1):
                return None
            uu = (dx // (p + 1), dy // (p + 1))
            if abs(uu[0]) > 1 or abs(uu[1]) > 1:
                return None
            if u is None:
                u = uu
            elif u != uu:
                return None
        info.append(u)
    plus = [h for h in range(NH) if info[h][0] == 1]
    minus = [h for h in range(NH) if info[h][0] == -1]
    zero = [h for h in range(NH) if info[h][0] == 0]
    if len(plus) != 3 or len(minus) != 3 or len(zero) != 2:
        return None
    hord = plus + minus + zero
    uys = [info[h][1] for h in hord]
    uxs = [info[h][0] for h in hord]
    return hord, uys, uxs


def _build_smats():
    """smat[k] for k in -4..4: m[j_in, j_out] = 1 at j_in = j_out + k."""
    mats = np.zeros((9, P, P), np.float32)
    for k in range(-4, 5):
        m = mats[k + 4]
        for j_out in range(W):
            j_in = j_out + k
            if 0 <= j_in < W:
                m[j_in, j_out] = 1.0
    return mats


def _np_reference(x, v_w, v_b, aw_w, aw_b, off_w, off_b, proj_w, proj_b, Hh, Ww):
    """Pure-numpy fallback mirroring reference.py (only used off-spec)."""
    B, N, C = x.shape
    v = (x @ v_w.T + v_b).reshape(B, N, NH, HD).transpose(0, 2, 1, 3)
    v = v.reshape(B * NH, Hh, Ww, HD)
    mh, mw = np.meshgrid(np.arange(Hh, dtype=x.dtype), np.arange(Ww, dtype=x.dtype),
                         indexing="ij")
    ref = np.stack([mw, mh], -1).reshape(1, N, 1, 2)
    off = (x @ off_w.T + off_b).reshape(B, N, NH, NP, 2).transpose(0, 2, 1, 3, 4)
    off = off.reshape(B * NH, N, NP, 2)
    grid = ref + off
    w = (x @ aw_w.T + aw_b).reshape(B, N, NH, NP).transpose(0, 2, 1, 3)
    w = w.reshape(B * NH, N, NP)
    w = np.exp(w - w.max(-1, keepdims=True))
    w = w / w.sum(-1, keepdims=True)
    G = B * NH
    vf = v.reshape(G, Hh * Ww, HD)
    gx, gy = grid[..., 0], grid[..., 1]
    x0 = np.floor(gx); y0 = np.floor(gy)
    wx1 = gx - x0; wx0 = 1.0 - wx1
    wy1 = gy - y0; wy0 = 1.0 - wy1
    x0i = x0.astype(np.int64); y0i = y0.astype(np.int64)

    def gather(xi, yi):
        valid = (xi >= 0) & (xi < Ww) & (yi >= 0) & (yi < Hh)
        idx = (np.clip(yi, 0, Hh - 1) * Ww + np.clip(xi, 0, Ww - 1))
        g = np.take_along_axis(vf, idx.reshape(G, -1, 1), axis=1)
        return g.reshape(*xi.shape, HD) * valid[..., None]

    samp = ((wy0 * wx0)[..., None] * gather(x0i, y0i)
            + (wy0 * wx1)[..., None] * gather(x0i + 1, y0i)
            + (wy1 * wx0)[..., None] * gather(x0i, y0i + 1)
            + (wy1 * wx1)[..., None] * gather(x0i + 1, y0i + 1))
    out = np.einsum("gnpd,gnp->gnd", samp, w)
    out = out.reshape(B, NH, N, HD).transpose(0, 2, 1, 3).reshape(B, N, C)
    return (out @ proj_w.T + proj_b).astype(np.float32)


# ---------------------------------------------------------------------------
# device program
# ---------------------------------------------------------------------------

def _build_program(uys):
    import concourse.bass as bass
    import concourse.mybir as mybir
    import concourse.tile as tile
    from concourse import bacc

    dt = mybir.dt
    f32 = dt.float32
    bf16 = dt.bfloat16

    # engine placement knobs. PSUM evacuations may only use scalar/vector
    # (GPSIMD cannot access PSUM on HW); pool takes SBUF-only weighting ops.
    EV_A = os.environ.get("EV_A", "sv")        # rotation for A evacs
    EV_OUT = os.environ.get("EV_OUT", "s")     # rotation for out evacs
    EV_OT0 = os.environ.get("EV_OT0", "s")     # D0 ot evacs
    EV_OT1 = os.environ.get("EV_OT1", "v")     # D1 ot evacs
    EV_Y0 = os.environ.get("EV_Y0", "sv")      # D0 y evacs (mc0, mc1)
    EV_Y1 = os.environ.get("EV_Y1", "ss")      # D1 y evacs
    WEIGHT_ROT = os.environ.get("WEIGHT_ROT", "vvvp")  # weighting ops rotation
    ENG = {"s": "scalar", "v": "vector", "p": "pool"}

    nc = bacc.Bacc("TRN2", target_bir_lowering=False, debug=False,
                   num_devices=N_CORES)

    _rotc = {}

    def evac(which):
        if len(which) > 1:  # rotation string like "sv"
            i = _rotc[which] = (_rotc.get(which, -1) + 1) % len(which)
            which = which[i]
        return {"s": nc.scalar.copy, "v": nc.vector.tensor_copy,
                "scalar": nc.scalar.copy, "vector": nc.vector.tensor_copy,
                "pool": nc.gpsimd.tensor_copy}[which]

    def tt_eng(rot):
        i = _rotc[rot] = (_rotc.get(rot, -1) + 1) % len(rot)
        return {"v": nc.vector, "p": nc.gpsimd, "s": None}[rot[i]]

    INTERLEAVE_C = os.environ.get("INTERLEAVE_C", "0") == "1"

    # ---- DRAM I/O ----
    xt_d = nc.dram_tensor("xt_dev", [DIM, TOK_V], bf16, kind="ExternalInput")
    wb_d = nc.dram_tensor("wb_cat", [2, P, NCH], bf16, kind="ExternalInput")
    s_d = nc.dram_tensor("s_mats", [9, P, P], bf16, kind="ExternalInput")
    pj_d = nc.dram_tensor("proj_t", [2, 2, P, P], bf16, kind="ExternalInput")
    id_d = nc.dram_tensor("ident", [P, P], bf16, kind="ExternalInput")
    y_d = [nc.dram_tensor(f"y{mc}", [P, ROWS_OUT * W], bf16,
                          kind="ExternalOutput") for mc in range(2)]

    NG = 9  # x DMA groups of 8 rows

    with tile.TileContext(nc) as tc:
        with (
            tc.tile_pool(name="const", bufs=1) as cpool,
            tc.tile_pool(name="big", bufs=1) as bigpool,
            tc.tile_pool(name="stA", bufs=2) as stA,
            tc.tile_pool(name="stB", bufs=2) as stB,
            tc.tile_pool(name="stM", bufs=1) as stM,
            tc.tile_pool(name="stD", bufs=2) as stD,
            tc.tile_pool(name="psA", bufs=2, space="PSUM") as psA,
            tc.tile_pool(name="psC", bufs=2, space="PSUM") as psC,
        ):
            # ---- constants (only wb gates phase A; rest loaded later).
            # wb is split per contraction half so the first matmul only
            # waits for its own half's DMA. ----
            wb_sb = cpool.tile([P, 2, NCH], bf16, tag="wb")
            wb_kc = [wb_sb[:, 0, :], wb_sb[:, 1, :]]

            def load_wb():
                nc.sync.dma_start(wb_sb[:],
                                  wb_d.rearrange("kc k f -> k kc f"))
            s_sb = cpool.tile([P, 9, P], bf16, tag="smats")
            pj_sb = cpool.tile([P, 2, 2, P], bf16, tag="proj")
            id_sb = cpool.tile([P, P], bf16, tag="ident")

            def load_late_consts():
                nc.sync.dma_start(s_sb[:], s_d.rearrange("s k f -> k s f"))
                nc.sync.dma_start(pj_sb[:],
                                  pj_d.rearrange("kc m k f -> k kc m f"))
                nc.sync.dma_start(id_sb[:], id_d[:])

            def sf(dx):  # forward shift matrix index
                return dx + 4

            def sb(dx):  # backward (S_-dx) index
                return -dx + 4

            # ---- persistent tiles ----
            # VL: [j, 288 ch, 72 row-slots]; ch = h'*32+d for v, 256+h'*4+p logits
            vl = bigpool.tile([P, NCH, ROWS_V], bf16, tag="VL")
            es = [bigpool.tile([P, NH * NP, 32], bf16, tag="E", name=f"e{h}")
                  for h in range(2)]
            eps = [bigpool.tile([P, 8, 3, 32], bf16, tag="EP", name=f"ep{h}")
                   for h in range(2)]
            outs = [bigpool.tile([P, DIM, 32], bf16, tag="OUT", name=f"out{h}")
                    for h in range(2)]

            def phase_a_dma(row0, nrows):
                xt_g = [stA.tile([P, 2048], bf16, tag=f"xt{kc}", bufs=3,
                                 name=f"xtg{kc}") for kc in range(2)]
                for kc in range(2):
                    nc.sync.dma_start(
                        xt_g[kc][:, :P * nrows],
                        xt_d[P * kc:P * kc + P,
                             P * row0:P * (row0 + nrows)])
                return xt_g

            def phase_a(row0, nrows, early, after_dma=None, xt_g=None):
                """x rows row0..row0+nrows: fused v+logit projection.

                Single-row PSUM tiles with a 4-deep ring: the evac round-trip
                (copy + 2 sem hops ~= 850ns) hides under 4 rows of PE work."""
                if xt_g is None:
                    xt_g = phase_a_dma(row0, nrows)
                if after_dma is not None:
                    after_dma()
                for rl in range(nrows):
                    a_ps = psA.tile([P, 512], f32, tag="a", bufs=4)
                    for kc in range(2):
                        nc.tensor.matmul(
                            a_ps[:, :NCH],
                            xt_g[kc][:, P * rl:P * rl + P],
                            wb_kc[kc][:], start=(kc == 0),
                            stop=(kc == 1))
                    s0 = row0 + rl
                    evac(EV_A)(vl[:, :, s0], a_ps[:, :NCH])

            def phase_b(half):
                """exp + softmax over the 4 points, all heads, 32 rows."""
                rr = 32 * half
                e_sb = es[half]
                nc.scalar.activation(
                    e_sb[:], vl[:, DIM:, HALO + rr:HALO + rr + 32],
                    mybir.ActivationFunctionType.Exp)
                ev = e_sb[:].rearrange("j (h p) i -> j h p i", p=NP)
                z0 = stB.tile([P, NH, 32], f32, tag="z0")
                z1 = stB.tile([P, NH, 32], f32, tag="z1")
                zr = stB.tile([P, NH, 32], f32, tag="zr")
                zb = stB.tile([P, NH, 32], bf16, tag="zb")
                nc.vector.tensor_tensor(z0[:], ev[:, :, 0, :], ev[:, :, 1, :],
                                        op=mybir.AluOpType.add)
                nc.vector.tensor_tensor(z1[:], ev[:, :, 2, :], ev[:, :, 3, :],
                                        op=mybir.AluOpType.add)
                nc.vector.tensor_tensor(z0[:], z0[:], z1[:],
                                        op=mybir.AluOpType.add)
                nc.vector.reciprocal(zr[:], z0[:])
                nc.vector.tensor_copy(zb[:], zr[:])
                nc.vector.tensor_tensor(
                    ev[:], ev[:],
                    zb[:].unsqueeze(2).broadcast_to([P, NH, NP, 32]),
                    op=mybir.AluOpType.mult)

            def phase_ep(half):
                """column-shifted weights E' = S_-dx^T E, batched 3 heads/dx."""
                e_sb = es[half]
                for rnd in range(2):           # rnd 0: dx=+1..+4, 1: dx=-1..-4
                    sgn = 1 if rnd == 0 else -1
                    hoff = 0 if rnd == 0 else 3
                    ep_ps = psC.tile([P, 4, 128], f32, tag="ep", bufs=1)
                    for k in range(1, 5):
                        dx = sgn * k
                        # heads hoff..hoff+2, point k-1: slots h'*4+(k-1)
                        rhs = e_sb[:].rearrange("j (h p) i -> j h p i", p=NP)[
                            :, hoff:hoff + 3, k - 1, :]
                        nc.tensor.matmul(ep_ps[:, k - 1, :96],
                                         s_sb[:, sb(dx), :], rhs,
                                         start=True, stop=True)
                    nc.scalar.copy(
                        eps[half][:, 4 * rnd:4 * rnd + 4],
                        ep_ps[:, :, :96].rearrange("j k (t i) -> j k t i", t=3))

            def _weight_slice(half, hp, p):
                """weight row [P, 32] for (permuted head hp, point p)."""
                if hp < 6:
                    rnd = 0 if hp < 3 else 1
                    return eps[half][:, 4 * rnd + p, hp % 3, :]
                return es[half][:, NP * hp + p, :]

            WEIGHT_BATCH = os.environ.get("WEIGHT_BATCH", "0") == "1"

            def _c_weight(half, m_t, t, hp):
                """weighting multiplies for one head into m_t[:, t]."""
                rr = 32 * half
                uy = uys[hp]
                if WEIGHT_BATCH:
                    # one op per head: overlapping-window AP over p
                    s1 = rr + HALO + uy
                    v_ap = vl[:, HD * hp:HD * hp + HD, s1:s1 + 32] \
                        .unsqueeze(1).broadcast_to([P, NP, HD, 32])
                    v_ap.ap[1] = [uy, NP]
                    w_ap = _weight_slice(half, hp, 0) \
                        .unsqueeze(1).unsqueeze(1) \
                        .broadcast_to([P, NP, HD, 32])
                    w_ap.ap[1] = [96, NP] if hp < 6 else [32, NP]
                    tt_eng(WEIGHT_ROT).tensor_tensor(
                        m_t[:, t], v_ap, w_ap, op=mybir.AluOpType.mult)
                    return
                for p in range(NP):
                    s0 = rr + HALO + uy * (p + 1)
                    tt_eng(WEIGHT_ROT).tensor_tensor(
                        m_t[:, t, p],
                        vl[:, HD * hp:HD * hp + HD, s0:s0 + 32],
                        _weight_slice(half, hp, p)
                        .unsqueeze(1).broadcast_to([P, HD, 32]),
                        op=mybir.AluOpType.mult)

            def _c_shift(half, m_t, t, hp, out_eng):
                """shift-accumulate + out evac for one head."""
                dxu = 1 if hp < 3 else (-1 if hp < 6 else 0)
                o_ps = [psC.tile([P, 512], f32, tag="o", bufs=3,
                                 name=f"o{ch}") for ch in range(2)]
                for p in range(NP):
                    dx = dxu * (p + 1)
                    mv = m_t[:, t, p].rearrange("j d i -> j (d i)")
                    for ch in range(2):
                        nc.tensor.matmul(
                            o_ps[ch][:], s_sb[:, sf(dx), :],
                            mv[:, 512 * ch:512 * ch + 512],
                            start=(p == 0), stop=(p == NP - 1))
                for ch in range(2):
                    evac(out_eng or EV_OUT)(
                        outs[half][:, HD * hp + 16 * ch:HD * hp + 16 * ch + 16, :],
                        o_ps[ch][:].rearrange("j (d i) -> j d i", d=16))

            def phase_c_heads(half, heads, mtag, out_eng=None):
                """weighting + shift-accumulate + evac for a group of heads."""
                m_t = stM.tile([P, 3, NP, 32, 32], bf16,
                               tag=mtag, bufs=1, name=mtag)
                if INTERLEAVE_C:
                    for t, hp in enumerate(heads):
                        _c_weight(half, m_t, t, hp)
                        _c_shift(half, m_t, t, hp, out_eng)
                else:
                    for t, hp in enumerate(heads):
                        _c_weight(half, m_t, t, hp)
                    for t, hp in enumerate(heads):
                        _c_shift(half, m_t, t, hp, out_eng)

            # ---- phase D: software-pipelined (trans(g+1) emitted before
            # proj(g) so PE never stalls on the ot evacuation copy) ----
            d_state = {}

            def d_trans(half, gl, ot_eng=None):
                ot_eng = ot_eng or (EV_OT0 if half == 0 else EV_OT1)
                """transposes of out-row group gl to channel-major."""
                i0 = 4 * gl
                ot_ps = psA.tile([P, 8, P], bf16, tag="a",
                                 name="otp", bufs=4)
                for kc in range(2):
                    for il in range(4):
                        nc.tensor.transpose(
                            ot_ps[:, 4 * kc + il, :],
                            outs[half][:, P * kc:P * kc + P, i0 + il],
                            id_sb[:])
                ot_sb = stD.tile([P, 2, 512], bf16, tag="ot", bufs=2)
                evac(ot_eng)(
                    ot_sb[:].rearrange("j kc f -> j (kc f)"),
                    ot_ps[:].rearrange("j a b -> j (a b)"))
                d_state[(half, gl)] = ot_sb

            def d_proj(half, gl, y_engs=None):
                """output projection + y evac (+DMA every 4th group)."""
                y_engs = y_engs or (EV_Y0 if half == 0 else EV_Y1)
                g = 8 * half + gl
                ot_sb = d_state.pop((half, gl))
                y_ps = [psA.tile([P, 512], f32, tag="a",
                                 name=f"yps{mc}", bufs=4) for mc in range(2)]
                for mc in range(2):
                    for kc in range(2):
                        nc.tensor.matmul(
                            y_ps[mc][:], pj_sb[:, kc, mc, :],
                            ot_sb[:, kc, :],
                            start=(kc == 0), stop=(kc == 1))
                q = gl % 2
                ysb = d_state.get(("ysb", half, gl // 2))
                if ysb is None:
                    ysb = stD.tile([P, 2, 2, 512], bf16, tag="y", bufs=3)
                    d_state[("ysb", half, gl // 2)] = ysb
                for mc in range(2):
                    evac(y_engs[mc])(ysb[:, mc, q, :], y_ps[mc][:])
                if q == 1:
                    g0 = 512 * (g - 1)
                    for mc in range(2):
                        nc.sync.dma_start(
                            y_d[mc][:, g0:g0 + 1024], ysb[:, mc, :, :])

            # ---- emission ----
            phase_a(0, 8, True, after_dma=load_wb)
            phase_a(8, 8, True)
            phase_a(16, 16, True)
            load_late_consts()
            phase_a(32, 16, True)
            phase_b(0)           # no PE work; overlaps A tail
            phase_a(48, 16, False)
            phase_ep(0)
            phase_c_heads(0, [0, 1, 2], "m0")
            phase_a(64, 4, False)
            phase_c_heads(0, [3, 4, 5], "m1")
            xt68 = phase_a_dma(68, 4)   # prefetch halo rows
            phase_c_heads(0, [6, 7], "m0", "sv")
            phase_a(68, 4, False, xt_g=xt68)  # fills the duo->B1 gap
            phase_b(1)
            d_trans(0, 0)   # fill the B1 latency with D0 groups
            d_trans(0, 1); d_proj(0, 0)
            d_trans(0, 2); d_proj(0, 1)
            phase_ep(1)
            d_trans(0, 3); d_proj(0, 2)
            phase_c_heads(1, [0, 1, 2], "m1")
            d_trans(0, 4); d_proj(0, 3)
            d_trans(0, 5); d_proj(0, 4)
            phase_c_heads(1, [3, 4, 5], "m0")
            d_trans(0, 6); d_proj(0, 5)
            d_trans(0, 7); d_proj(0, 6)
            phase_c_heads(1, [6, 7], "m1", "sv")
            d_proj(0, 7)
            d_trans(1, 0)
            for gl in range(1, 8):
                d_trans(1, gl); d_proj(1, gl - 1)
            d_proj(1, 7)

    nc.compile()
    return nc


# ---------------------------------------------------------------------------
# host wrapper
# ---------------------------------------------------------------------------

def kernel(x, v_w, v_b, aw_w, aw_b, off_w, off_b, proj_w, proj_b, H=128, W=128,
           **_unused):
    import ml_dtypes
    bf = ml_dtypes.bfloat16

    x = np.ascontiguousarray(np.asarray(x, np.float32))
    v_w = np.asarray(v_w, np.float32); v_b = np.asarray(v_b, np.float32)
    aw_w = np.asarray(aw_w, np.float32); aw_b = np.asarray(aw_b, np.float32)
    off_w = np.asarray(off_w, np.float32); off_b = np.asarray(off_b, np.float32)
    proj_w = np.asarray(proj_w, np.float32); proj_b = np.asarray(proj_b, np.float32)

    geom = _derive_geometry(off_b)
    if (np.any(off_w != 0.0) or int(H) != 128 or int(W) != 128 or geom is None
            or np.any(v_b) or np.any(aw_b) or np.any(proj_b)):
        return _np_reference(x, v_w, v_b, aw_w, aw_b, off_w, off_b,
                             proj_w, proj_b, int(H), int(W))
    hord, uys, _uxs = geom

    key = ("prog2", tuple(uys))
    if key not in _cache:
        _cache[key] = _build_program(uys)
    nc = _cache[key]

    B = x.shape[0]
    # ---- host prep (shared across cores) ----
    # channel permutation: ch' = h'*32+d  <-  orig h*32+d
    vperm = np.concatenate([np.arange(HD) + HD * h for h in hord])
    aperm = np.concatenate([np.arange(NP) + NP * h for h in hord])
    wb_cat = np.empty((2, P, NCH), np.float32)
    v_wp = v_w[vperm]          # [256 out-ch', 256 in]
    aw_wp = aw_w[aperm]        # [32 out-ch', 256 in]
    for kc in range(2):
        wb_cat[kc, :, :DIM] = v_wp[:, P * kc:P * (kc + 1)].T
        wb_cat[kc, :, DIM:] = aw_wp[:, P * kc:P * (kc + 1)].T
    pj_perm = proj_w[:, vperm]  # permute contraction columns
    pj_t = np.empty((2, 2, P, P), np.float32)
    for kc in range(2):
        for mc in range(2):
            pj_t[kc, mc] = pj_perm[P * mc:P * (mc + 1), P * kc:P * (kc + 1)].T
    shared = dict(wb_cat=np.ascontiguousarray(wb_cat).astype(bf),
                  s_mats=np.ascontiguousarray(_build_smats()).astype(bf),
                  proj_t=np.ascontiguousarray(pj_t).astype(bf),
                  ident=np.eye(P, dtype=np.float32).astype(bf))

    xr = x.reshape(B, H, W, DIM)
    in_maps = []
    for d in range(N_CORES):
        b, half = d // 2, d % 2
        r0 = ROWS_OUT * half
        x_dev = np.zeros((ROWS_V, W, DIM), np.float32)
        lo, hi = max(0, r0 - HALO), min(H, r0 + ROWS_OUT + HALO)
        x_dev[lo - (r0 - HALO):hi - (r0 - HALO)] = xr[b, lo:hi]
        m = dict(shared)
        m["xt_dev"] = np.ascontiguousarray(
            x_dev.reshape(TOK_V, DIM).T).astype(bf)
        in_maps.append(m)

    from concourse import bass_utils
    res = bass_utils.run_bass_kernel_spmd(
        nc, in_maps, core_ids=list(range(N_CORES)),
        trace=os.environ.get("KERNEL_TRACE", "0") == "1")
    kernel.last_results = res

    y = np.empty((B, N_TOK, DIM), np.float32)
    for d in range(N_CORES):
        b, half = d // 2, d % 2
        yd = np.concatenate([np.asarray(res.results[d]["y0"]),
                             np.asarray(res.results[d]["y1"])], 0)
        y[b, ROWS_OUT * W * half:ROWS_OUT * W * (half + 1), :] = \
            yd.astype(np.float32).T
    return y


# revision 61
# speedup vs baseline: 1.5702x; 1.0070x over previous
"""Trainium2 Bass kernel for nn_Attention_34840774705279 (sparse/deformable attention).

Math (matches reference.py):
  v   = x @ v_w.T                  -> per-head maps [B*NH, H, W, HD]
  off = off_b (off_w == 0)         -> constant integer offsets (dx, dy) = p*(ux, uy)
  w   = softmax_p(x @ aw_w.T)
  out[i,j] = sum_p w_p[i,j] * v[i+dy_p, j+dx_p]   (zero outside the map)
  y   = out @ proj_w.T

Sharding (8 cores, uniform SPMD): core d -> batch b = d//2, row-half r0 =
64*(d%2); each core computes all 8 heads for its 64 output rows using a 4-row
halo of v rows (host zero-pads), so no cross-core traffic; host concatenates.

v2 design (all bf16 data path, f32 PSUM accumulation):
  A. pixel-major projection: per image row r, a_ps[j, 288] = x_row^T @ [v|aw]
     (2 matmuls, contraction 256); pairs of rows share one PSUM tile and one
     ScalarE evacuation copy into VL[j, 288ch, 72slots] (slot innermost so the
     DVE weighting op later hits its 2-byte fast path).
  B. softmax over points: one exp (ScalarE), adds + reciprocal (DVE, f32),
     one batched normalize multiply (DVE bf16 2x mode).
  C. weight-then-shift: heads are host-permuted into [+dx trio, -dx trio,
     dx=0 duo] so the column-shifted weights E' = S_-dx^T E are computed with
     ONE matmul per dx value (3 heads batched); DVE multiplies the V window by
     E' (bf16 2x); 0/1 shift matrices S_dx matmul-accumulate the 4 points in
     PSUM (the p-sum rides the PSUM accumulation for free).
  D. output projection: PE transposes OUT rows to channel-major (bf16 PSUM),
     y^T = proj^T @ OUT^T; y leaves as bf16, host casts + transposes.
  Evac copies are spread over ScalarE/Pool(GpSimd)/DVE to balance engines.
"""

import os
import sys
import math

import numpy as np

sys.path.insert(0, "/opt/trn_rl_repo")

P = 128
H = W = 128
NH, NP, HD = 8, 4, 32
DIM = 256
N_TOK = H * W
ROWS_OUT = 64          # output rows per core
HALO = 4
ROWS_V = ROWS_OUT + 2 * HALO   # 72 v-row slots per core
TOK_V = ROWS_V * W             # 9216
N_CORES = 8
NCH = DIM + NH * NP    # 288 channels out of the fused projection

_cache = {}


# ---------------------------------------------------------------------------
# geometry: constant offsets -> head permutation + shift matrices
# ---------------------------------------------------------------------------

def _derive_geometry(off_b):
    """For each head h expect offsets (dx, dy) = p*(ux, uy), ux/uy in {-1,0,1},
    integer (bilinear weight ~1). Returns (hord, uys, uxs) with heads permuted
    to [ux=+1 trio, ux=-1 trio, ux=0 duo], or None if the pattern fails."""
    ob = np.asarray(off_b, np.float64).reshape(NH, NP, 2)
    info = []
    for h in range(NH):
        u = None
        for p in range(NP):
            fx, fy = ob[h, p, 0], ob[h, p, 1]
            dx, dy = round(fx), round(fy)
            # must be integer shifts with negligible bilinear remainder
            if abs(fx - dx) > 1e-6 or abs(fy - dy) > 1e-6:
                return None
            if dx % (p + 1) or dy % (p + 1):
                return None
            uu = (dx // (p + 1), dy // (p + 1))
            if abs(uu[0]) > 1 or abs(uu[1]) > 1:
                return None
            if u is None:
                u = uu
            elif u != uu:
                return None
        info.append(u)
    plus = [h for h in range(NH) if info[h][0] == 1]
    minus = [h for h in range(NH) if info[h][0] == -1]
    zero = [h for h in range(NH) if info[h][0] == 0]
    if len(plus) != 3 or len(minus) != 3 or len(zero) != 2:
        return None
    hord = plus + minus + zero
    uys = [info[h][1] for h in hord]
    uxs = [info[h][0] for h in hord]
    return hord, uys, uxs


def _build_smats():
    """smat[k] for k in -4..4: m[j_in, j_out] = 1 at j_in = j_out + k."""
    mats = np.zeros((9, P, P), np.float32)
    for k in range(-4, 5):
        m = mats[k + 4]
        for j_out in range(W):
            j_in = j_out + k
            if 0 <= j_in < W:
                m[j_in, j_out] = 1.0
    return mats


def _np_reference(x, v_w, v_b, aw_w, aw_b, off_w, off_b, proj_w, proj_b, Hh, Ww):
    """Pure-numpy fallback mirroring reference.py (only used off-spec)."""
    B, N, C = x.shape
    v = (x @ v_w.T + v_b).reshape(B, N, NH, HD).transpose(0, 2, 1, 3)
    v = v.reshape(B * NH, Hh, Ww, HD)
    mh, mw = np.meshgrid(np.arange(Hh, dtype=x.dtype), np.arange(Ww, dtype=x.dtype),
                         indexing="ij")
    ref = np.stack([mw, mh], -1).reshape(1, N, 1, 2)
    off = (x @ off_w.T + off_b).reshape(B, N, NH, NP, 2).transpose(0, 2, 1, 3, 4)
    off = off.reshape(B * NH, N, NP, 2)
    grid = ref + off
    w = (x @ aw_w.T + aw_b).reshape(B, N, NH, NP).transpose(0, 2, 1, 3)
    w = w.reshape(B * NH, N, NP)
    w = np.exp(w - w.max(-1, keepdims=True))
    w = w / w.sum(-1, keepdims=True)
    G = B * NH
    vf = v.reshape(G, Hh * Ww, HD)
    gx, gy = grid[..., 0], grid[..., 1]
    x0 = np.floor(gx); y0 = np.floor(gy)
    wx1 = gx - x0; wx0 = 1.0 - wx1
    wy1 = gy - y0; wy0 = 1.0 - wy1
    x0i = x0.astype(np.int64); y0i = y0.astype(np.int64)

    def gather(xi, yi):
        valid = (xi >= 0) & (xi < Ww) & (yi >= 0) & (yi < Hh)
        idx = (np.clip(yi, 0, Hh - 1) * Ww + np.clip(xi, 0, Ww - 1))
        g = np.take_along_axis(vf, idx.reshape(G, -1, 1), axis=1)
        return g.reshape(*xi.shape, HD) * valid[..., None]

    samp = ((wy0 * wx0)[..., None] * gather(x0i, y0i)
            + (wy0 * wx1)[..., None] * gather(x0i + 1, y0i)
            + (wy1 * wx0)[..., None] * gather(x0i, y0i + 1)
            + (wy1 * wx1)[..., None] * gather(x0i + 1, y0i + 1))
    out = np.einsum("gnpd,gnp->gnd", samp, w)
    out = out.reshape(B, NH, N, HD).transpose(0, 2, 1, 3).reshape(B, N, C)
    return (out @ proj_w.T + proj_b).astype(np.float32)


# ---------------------------------------------------------------------------
# device program
# ---------------------------------------------------------------------------

def _build_program(uys):
    import concourse.bass as bass
    import concourse.mybir as mybir
    import concourse.tile as tile
    from concourse import bacc

    dt = mybir.dt
    f32 = dt.float32
    bf16 = dt.bfloat16

    # engine placement knobs. PSUM evacuations may only use scalar/vector
    # (GPSIMD cannot access PSUM on HW); pool takes SBUF-only weighting ops.
    EV_A = os.environ.get("EV_A", "sv")        # rotation for A evacs
    EV_OUT = os.environ.get("EV_OUT", "s")     # rotation for out evacs
    EV_OT0 = os.environ.get("EV_OT0", "s")     # D0 ot evacs
    EV_OT1 = os.environ.get("EV_OT1", "v")     # D1 ot evacs
    EV_Y0 = os.environ.get("EV_Y0", "sv")      # D0 y evacs (mc0, mc1)
    EV_Y1 = os.environ.get("EV_Y1", "ss")      # D1 y evacs
    WEIGHT_ROT = os.environ.get("WEIGHT_ROT", "vvvp")  # weighting ops rotation
    ENG = {"s": "scalar", "v": "vector", "p": "pool"}

    nc = bacc.Bacc("TRN2", target_bir_lowering=False, debug=False,
                   num_devices=N_CORES)

    _rotc = {}

    def evac(which):
        if len(which) > 1:  # rotation string like "sv"
            i = _rotc[which] = (_rotc.get(which, -1) + 1) % len(which)
            which = which[i]
        return {"s": nc.scalar.copy, "v": nc.vector.tensor_copy,
                "scalar": nc.scalar.copy, "vector": nc.vector.tensor_copy,
                "pool": nc.gpsimd.tensor_copy}[which]

    def tt_eng(rot):
        i = _rotc[rot] = (_rotc.get(rot, -1) + 1) % len(rot)
        return {"v": nc.vector, "p": nc.gpsimd, "s": None}[rot[i]]

    INTERLEAVE_C = os.environ.get("INTERLEAVE_C", "0") == "1"

    # ---- DRAM I/O ----
    xt_d = nc.dram_tensor("xt_dev", [DIM, TOK_V], bf16, kind="ExternalInput")
    wb_d = nc.dram_tensor("wb_cat", [2, P, NCH], bf16, kind="ExternalInput")
    s_d = nc.dram_tensor("s_mats", [9, P, P], bf16, kind="ExternalInput")
    pj_d = nc.dram_tensor("proj_t", [2, 2, P, P], bf16, kind="ExternalInput")
    id_d = nc.dram_tensor("ident", [P, P], bf16, kind="ExternalInput")
    y_d = [nc.dram_tensor(f"y{mc}", [P, ROWS_OUT * W], bf16,
                          kind="ExternalOutput") for mc in range(2)]

    NG = 9  # x DMA groups of 8 rows

    with tile.TileContext(nc) as tc:
        with (
            tc.tile_pool(name="const", bufs=1) as cpool,
            tc.tile_pool(name="big", bufs=1) as bigpool,
            tc.tile_pool(name="stA", bufs=2) as stA,
            tc.tile_pool(name="stB", bufs=2) as stB,
            tc.tile_pool(name="stM", bufs=1) as stM,
            tc.tile_pool(name="stD", bufs=2) as stD,
            tc.tile_pool(name="psA", bufs=2, space="PSUM") as psA,
            tc.tile_pool(name="psC", bufs=2, space="PSUM") as psC,
        ):
            # ---- constants (only wb gates phase A; rest loaded later).
            # wb is split per contraction half so the first matmul only
            # waits for its own half's DMA. ----
            wb_sb = cpool.tile([P, 2, NCH], bf16, tag="wb")
            wb_kc = [wb_sb[:, 0, :], wb_sb[:, 1, :]]

            def load_wb():
                nc.sync.dma_start(wb_sb[:],
                                  wb_d.rearrange("kc k f -> k kc f"))
            s_sb = cpool.tile([P, 9, P], bf16, tag="smats")
            pj_sb = cpool.tile([P, 2, 2, P], bf16, tag="proj")
            id_sb = cpool.tile([P, P], bf16, tag="ident")

            def load_late_consts():
                nc.sync.dma_start(s_sb[:], s_d.rearrange("s k f -> k s f"))
                nc.sync.dma_start(pj_sb[:],
                                  pj_d.rearrange("kc m k f -> k kc m f"))
                nc.sync.dma_start(id_sb[:], id_d[:])

            def sf(dx):  # forward shift matrix index
                return dx + 4

            def sb(dx):  # backward (S_-dx) index
                return -dx + 4

            # ---- persistent tiles ----
            # VL: [j, 288 ch, 72 row-slots]; ch = h'*32+d for v, 256+h'*4+p logits
            vl = bigpool.tile([P, NCH, ROWS_V], bf16, tag="VL")
            es = [bigpool.tile([P, NH * NP, 32], bf16, tag="E", name=f"e{h}")
                  for h in range(2)]
            eps = [bigpool.tile([P, 8, 3, 32], bf16, tag="EP", name=f"ep{h}")
                   for h in range(2)]
            outs = [bigpool.tile([P, DIM, 32], bf16, tag="OUT", name=f"out{h}")
                    for h in range(2)]

            def phase_a_dma(row0, nrows, mid=None):
                xt_g = [stA.tile([P, 2048], bf16, tag=f"xt{kc}", bufs=3,
                                 name=f"xtg{kc}") for kc in range(2)]
                for kc in range(2):
                    nc.sync.dma_start(
                        xt_g[kc][:, :P * nrows],
                        xt_d[P * kc:P * kc + P,
                             P * row0:P * (row0 + nrows)])
                    if kc == 0 and mid is not None:
                        mid()
                return xt_g

            def phase_a(row0, nrows, early, after_dma=None, xt_g=None):
                """x rows row0..row0+nrows: fused v+logit projection.

                Single-row PSUM tiles with a 4-deep ring: the evac round-trip
                (copy + 2 sem hops ~= 850ns) hides under 4 rows of PE work."""
                if xt_g is None:
                    xt_g = phase_a_dma(row0, nrows, mid=after_dma)
                for rl in range(nrows):
                    a_ps = psA.tile([P, 512], f32, tag="a", bufs=4)
                    for kc in range(2):
                        nc.tensor.matmul(
                            a_ps[:, :NCH],
                            xt_g[kc][:, P * rl:P * rl + P],
                            wb_kc[kc][:], start=(kc == 0),
                            stop=(kc == 1))
                    s0 = row0 + rl
                    evac(EV_A)(vl[:, :, s0], a_ps[:, :NCH])

            def phase_b(half, eng=None):
                """exp + softmax over the 4 points, all heads, 32 rows."""
                eng = eng or nc.vector
                rr = 32 * half
                e_sb = es[half]
                nc.scalar.activation(
                    e_sb[:], vl[:, DIM:, HALO + rr:HALO + rr + 32],
                    mybir.ActivationFunctionType.Exp)
                ev = e_sb[:].rearrange("j (h p) i -> j h p i", p=NP)
                z0 = stB.tile([P, NH, 32], f32, tag="z0")
                z1 = stB.tile([P, NH, 32], f32, tag="z1")
                zr = stB.tile([P, NH, 32], f32, tag="zr")
                zb = stB.tile([P, NH, 32], bf16, tag="zb")
                eng.tensor_tensor(z0[:], ev[:, :, 0, :], ev[:, :, 1, :],
                                  op=mybir.AluOpType.add)
                eng.tensor_tensor(z1[:], ev[:, :, 2, :], ev[:, :, 3, :],
                                  op=mybir.AluOpType.add)
                eng.tensor_tensor(z0[:], z0[:], z1[:],
                                  op=mybir.AluOpType.add)
                nc.vector.reciprocal(zr[:], z0[:])
                nc.vector.tensor_copy(zb[:], zr[:])
                eng.tensor_tensor(
                    ev[:], ev[:],
                    zb[:].unsqueeze(2).broadcast_to([P, NH, NP, 32]),
                    op=mybir.AluOpType.mult)

            def phase_ep(half):
                """column-shifted weights E' = S_-dx^T E, batched 3 heads/dx."""
                e_sb = es[half]
                for rnd in range(2):           # rnd 0: dx=+1..+4, 1: dx=-1..-4
                    sgn = 1 if rnd == 0 else -1
                    hoff = 0 if rnd == 0 else 3
                    ep_ps = psC.tile([P, 4, 128], f32, tag="ep", bufs=1)
                    for k in range(1, 5):
                        dx = sgn * k
                        # heads hoff..hoff+2, point k-1: slots h'*4+(k-1)
                        rhs = e_sb[:].rearrange("j (h p) i -> j h p i", p=NP)[
                            :, hoff:hoff + 3, k - 1, :]
                        nc.tensor.matmul(ep_ps[:, k - 1, :96],
                                         s_sb[:, sb(dx), :], rhs,
                                         start=True, stop=True)
                    evac(os.environ.get("EV_EP", "s"))(
                        eps[half][:, 4 * rnd:4 * rnd + 4],
                        ep_ps[:, :, :96].rearrange("j k (t i) -> j k t i", t=3))

            def _weight_slice(half, hp, p):
                """weight row [P, 32] for (permuted head hp, point p)."""
                if hp < 6:
                    rnd = 0 if hp < 3 else 1
                    return eps[half][:, 4 * rnd + p, hp % 3, :]
                return es[half][:, NP * hp + p, :]

            WEIGHT_BATCH = os.environ.get("WEIGHT_BATCH", "0") == "1"

            def _c_weight(half, m_t, t, hp):
                """weighting multiplies for one head into m_t[:, t]."""
                rr = 32 * half
                uy = uys[hp]
                if WEIGHT_BATCH:
                    # one op per head: overlapping-window AP over p
                    s1 = rr + HALO + uy
                    v_ap = vl[:, HD * hp:HD * hp + HD, s1:s1 + 32] \
                        .unsqueeze(1).broadcast_to([P, NP, HD, 32])
                    v_ap.ap[1] = [uy, NP]
                    w_ap = _weight_slice(half, hp, 0) \
                        .unsqueeze(1).unsqueeze(1) \
                        .broadcast_to([P, NP, HD, 32])
                    w_ap.ap[1] = [96, NP] if hp < 6 else [32, NP]
                    tt_eng(WEIGHT_ROT).tensor_tensor(
                        m_t[:, t], v_ap, w_ap, op=mybir.AluOpType.mult)
                    return
                for p in range(NP):
                    s0 = rr + HALO + uy * (p + 1)
                    tt_eng(WEIGHT_ROT).tensor_tensor(
                        m_t[:, t, p],
                        vl[:, HD * hp:HD * hp + HD, s0:s0 + 32],
                        _weight_slice(half, hp, p)
                        .unsqueeze(1).broadcast_to([P, HD, 32]),
                        op=mybir.AluOpType.mult)

            def _c_shift(half, m_t, t, hp, out_eng):
                """shift-accumulate + out evac for one head."""
                dxu = 1 if hp < 3 else (-1 if hp < 6 else 0)
                o_ps = [psC.tile([P, 512], f32, tag="o", bufs=3,
                                 name=f"o{ch}") for ch in range(2)]
                for p in range(NP):
                    dx = dxu * (p + 1)
                    mv = m_t[:, t, p].rearrange("j d i -> j (d i)")
                    for ch in range(2):
                        nc.tensor.matmul(
                            o_ps[ch][:], s_sb[:, sf(dx), :],
                            mv[:, 512 * ch:512 * ch + 512],
                            start=(p == 0), stop=(p == NP - 1))
                for ch in range(2):
                    evac(out_eng or EV_OUT)(
                        outs[half][:, HD * hp + 16 * ch:HD * hp + 16 * ch + 16, :],
                        o_ps[ch][:].rearrange("j (d i) -> j d i", d=16))

            def phase_c_heads(half, heads, mtag, out_eng=None):
                """weighting + shift-accumulate + evac for a group of heads."""
                m_t = stM.tile([P, 3, NP, 32, 32], bf16,
                               tag=mtag, bufs=1, name=mtag)
                if INTERLEAVE_C:
                    for t, hp in enumerate(heads):
                        _c_weight(half, m_t, t, hp)
                        _c_shift(half, m_t, t, hp, out_eng)
                else:
                    for t, hp in enumerate(heads):
                        _c_weight(half, m_t, t, hp)
                    for t, hp in enumerate(heads):
                        _c_shift(half, m_t, t, hp, out_eng)

            # ---- phase D: software-pipelined (trans(g+1) emitted before
            # proj(g) so PE never stalls on the ot evacuation copy) ----
            d_state = {}

            def d_trans(half, gl, ot_eng=None):
                ot_eng = ot_eng or (EV_OT0 if half == 0 else EV_OT1)
                """transposes of out-row group gl to channel-major."""
                i0 = 4 * gl
                ot_ps = psA.tile([P, 8, P], bf16, tag="a",
                                 name="otp", bufs=4)
                for kc in range(2):
                    for il in range(4):
                        nc.tensor.transpose(
                            ot_ps[:, 4 * kc + il, :],
                            outs[half][:, P * kc:P * kc + P, i0 + il],
                            id_sb[:])
                ot_sb = stD.tile([P, 2, 512], bf16, tag="ot", bufs=2)
                evac(ot_eng)(
                    ot_sb[:].rearrange("j kc f -> j (kc f)"),
                    ot_ps[:].rearrange("j a b -> j (a b)"))
                d_state[(half, gl)] = ot_sb

            def d_proj(half, gl, y_engs=None):
                """output projection + y evac (+DMA every 4th group)."""
                y_engs = y_engs or (EV_Y0 if half == 0 else EV_Y1)
                if half == 1 and gl >= 6:
                    y_engs = "sv"   # parallel evac on the critical tail
                g = 8 * half + gl
                ot_sb = d_state.pop((half, gl))
                y_ps = [psA.tile([P, 512], f32, tag="a",
                                 name=f"yps{mc}", bufs=4) for mc in range(2)]
                for mc in range(2):
                    for kc in range(2):
                        nc.tensor.matmul(
                            y_ps[mc][:], pj_sb[:, kc, mc, :],
                            ot_sb[:, kc, :],
                            start=(kc == 0), stop=(kc == 1))
                q = gl % 2
                ysb = d_state.get(("ysb", half, gl // 2))
                if ysb is None:
                    ysb = stD.tile([P, 2, 2, 512], bf16, tag="y", bufs=3)
                    d_state[("ysb", half, gl // 2)] = ysb
                for mc in range(2):
                    evac(y_engs[mc])(ysb[:, mc, q, :], y_ps[mc][:])
                if q == 1:
                    g0 = 512 * (g - 1)
                    for mc in range(2):
                        nc.sync.dma_start(
                            y_d[mc][:, g0:g0 + 1024], ysb[:, mc, :, :])

            # ---- emission ----
            phase_a(0, 8, True, after_dma=load_wb)
            phase_a(8, 8, True)
            phase_a(16, 16, True)
            load_late_consts()
            phase_a(32, 16, True)
            phase_b(0)           # no PE work; overlaps A tail
            phase_a(48, 16, False)
            phase_ep(0)
            phase_c_heads(0, [0, 1, 2], "m0")
            phase_a(64, 4, False)
            phase_c_heads(0, [3, 4, 5], "m1")
            xt68 = phase_a_dma(68, 4)   # prefetch halo rows
            phase_c_heads(0, [6, 7], "m0", "sv")
            phase_a(68, 4, False, xt_g=xt68)  # fills the duo->B1 gap
            phase_b(1)
            d_trans(0, 0)   # fill the B1 latency with D0 groups
            d_trans(0, 1); d_proj(0, 0)
            d_trans(0, 2); d_proj(0, 1)
            phase_ep(1)
            d_trans(0, 3); d_proj(0, 2)
            phase_c_heads(1, [0, 1, 2], "m1")
            d_trans(0, 4); d_proj(0, 3)
            d_trans(0, 5); d_proj(0, 4)
            phase_c_heads(1, [3, 4, 5], "m0")
            d_trans(0, 6); d_proj(0, 5)
            d_trans(0, 7); d_proj(0, 6)
            phase_c_heads(1, [6, 7], "m1", "sv")
            d_proj(0, 7)
            d_trans(1, 0)
            for gl in range(1, 8):
                d_trans(1, gl); d_proj(1, gl - 1)
            d_proj(1, 7)

    nc.compile()
    return nc


# ---------------------------------------------------------------------------
# host wrapper
# ---------------------------------------------------------------------------

def kernel(x, v_w, v_b, aw_w, aw_b, off_w, off_b, proj_w, proj_b, H=128, W=128,
           **_unused):
    import ml_dtypes
    bf = ml_dtypes.bfloat16

    x = np.ascontiguousarray(np.asarray(x, np.float32))
    v_w = np.asarray(v_w, np.float32); v_b = np.asarray(v_b, np.float32)
    aw_w = np.asarray(aw_w, np.float32); aw_b = np.asarray(aw_b, np.float32)
    off_w = np.asarray(off_w, np.float32); off_b = np.asarray(off_b, np.float32)
    proj_w = np.asarray(proj_w, np.float32); proj_b = np.asarray(proj_b, np.float32)

    geom = _derive_geometry(off_b)
    if (np.any(off_w != 0.0) or int(H) != 128 or int(W) != 128 or geom is None
            or np.any(v_b) or np.any(aw_b) or np.any(proj_b)):
        return _np_reference(x, v_w, v_b, aw_w, aw_b, off_w, off_b,
                             proj_w, proj_b, int(H), int(W))
    hord, uys, _uxs = geom

    key = ("prog2", tuple(uys))
    if key not in _cache:
        _cache[key] = _build_program(uys)
    nc = _cache[key]

    B = x.shape[0]
    # ---- host prep (shared across cores) ----
    # channel permutation: ch' = h'*32+d  <-  orig h*32+d
    vperm = np.concatenate([np.arange(HD) + HD * h for h in hord])
    aperm = np.concatenate([np.arange(NP) + NP * h for h in hord])
    wb_cat = np.empty((2, P, NCH), np.float32)
    v_wp = v_w[vperm]          # [256 out-ch', 256 in]
    aw_wp = aw_w[aperm]        # [32 out-ch', 256 in]
    for kc in range(2):
        wb_cat[kc, :, :DIM] = v_wp[:, P * kc:P * (kc + 1)].T
        wb_cat[kc, :, DIM:] = aw_wp[:, P * kc:P * (kc + 1)].T
    pj_perm = proj_w[:, vperm]  # permute contraction columns
    pj_t = np.empty((2, 2, P, P), np.float32)
    for kc in range(2):
        for mc in range(2):
            pj_t[kc, mc] = pj_perm[P * mc:P * (mc + 1), P * kc:P * (kc + 1)].T
    shared = dict(wb_cat=np.ascontiguousarray(wb_cat).astype(bf),
                  s_mats=np.ascontiguousarray(_build_smats()).astype(bf),
                  proj_t=np.ascontiguousarray(pj_t).astype(bf),
                  ident=np.eye(P, dtype=np.float32).astype(bf))

    xr = x.reshape(B, H, W, DIM)
    in_maps = []
    for d in range(N_CORES):
        b, half = d // 2, d % 2
        r0 = ROWS_OUT * half
        x_dev = np.zeros((ROWS_V, W, DIM), np.float32)
        lo, hi = max(0, r0 - HALO), min(H, r0 + ROWS_OUT + HALO)
        x_dev[lo - (r0 - HALO):hi - (r0 - HALO)] = xr[b, lo:hi]
        m = dict(shared)
        m["xt_dev"] = np.ascontiguousarray(
            x_dev.reshape(TOK_V, DIM).T).astype(bf)
        in_maps.append(m)

    from concourse import bass_utils
    res = bass_utils.run_bass_kernel_spmd(
        nc, in_maps, core_ids=list(range(N_CORES)),
        trace=os.environ.get("KERNEL_TRACE", "0") == "1")
    kernel.last_results = res

    y = np.empty((B, N_TOK, DIM), np.float32)
    for d in range(N_CORES):
        b, half = d // 2, d % 2
        yd = np.concatenate([np.asarray(res.results[d]["y0"]),
                             np.asarray(res.results[d]["y1"])], 0)
        y[b, ROWS_OUT * W * half:ROWS_OUT * W * (half + 1), :] = \
            yd.astype(np.float32).T
    return y


# revision 68
# speedup vs baseline: 1.5709x; 1.0004x over previous
"""Trainium2 Bass kernel for nn_Attention_34840774705279 (sparse/deformable attention).

Math (matches reference.py):
  v   = x @ v_w.T                  -> per-head maps [B*NH, H, W, HD]
  off = off_b (off_w == 0)         -> constant integer offsets (dx, dy) = p*(ux, uy)
  w   = softmax_p(x @ aw_w.T)
  out[i,j] = sum_p w_p[i,j] * v[i+dy_p, j+dx_p]   (zero outside the map)
  y   = out @ proj_w.T

Sharding (8 cores, uniform SPMD): core d -> batch b = d//2, row-half r0 =
64*(d%2); each core computes all 8 heads for its 64 output rows using a 4-row
halo of v rows (host zero-pads), so no cross-core traffic; host concatenates.

v2 design (all bf16 data path, f32 PSUM accumulation):
  A. pixel-major projection: per image row r, a_ps[j, 288] = x_row^T @ [v|aw]
     (2 matmuls, contraction 256); single-row PSUM tiles on a 4-deep ring so
     the evacuation round-trip hides under PE work; evac copies (alternating
     ScalarE/DVE) land in VL[j, 288ch, 72slots] with the row-slot innermost
     so the DVE weighting op later hits its 2-byte fast path.
  B. softmax over points: one exp (ScalarE), adds + reciprocal (DVE, f32),
     one batched normalize multiply (DVE bf16 2x mode).
  C. weight-then-shift: heads are host-permuted into [+dx trio, -dx trio,
     dx=0 duo] so the column-shifted weights E' = S_-dx^T E are computed with
     ONE matmul per dx value (3 heads batched); DVE/Pool multiply the V window
     by E' (bf16 2x on DVE; Pool takes every 4th op - it cannot touch PSUM so
     SBUF-only weighting is the one job it can absorb); 0/1 shift matrices
     S_dx matmul-accumulate the 4 points in PSUM (the p-sum rides the PSUM
     accumulation for free).
  D. output projection: PE transposes OUT rows to channel-major (bf16 PSUM),
     y^T = proj^T @ OUT^T; y leaves as bf16, host casts + transposes.
     Software-pipelined: group g+1's transposes are emitted before group g's
     proj matmuls so PE never stalls on the ot evacuation copy.
  Emission interleaves A's tail under C0, D0 under B1/ep1/C1, and splits the
  pure-halo x rows (68-71) out of phase A to fill the C0-duo -> B1 gap.
"""

import os
import sys
import math

import numpy as np

sys.path.insert(0, "/opt/trn_rl_repo")

P = 128
H = W = 128
NH, NP, HD = 8, 4, 32
DIM = 256
N_TOK = H * W
ROWS_OUT = 64          # output rows per core
HALO = 4
ROWS_V = ROWS_OUT + 2 * HALO   # 72 v-row slots per core
TOK_V = ROWS_V * W             # 9216
N_CORES = 8
NCH = DIM + NH * NP    # 288 channels out of the fused projection

_cache = {}


# ---------------------------------------------------------------------------
# geometry: constant offsets -> head permutation + shift matrices
# ---------------------------------------------------------------------------

def _derive_geometry(off_b):
    """For each head h expect offsets (dx, dy) = p*(ux, uy), ux/uy in {-1,0,1},
    integer (bilinear weight ~1). Returns (hord, uys, uxs) with heads permuted
    to [ux=+1 trio, ux=-1 trio, ux=0 duo], or None if the pattern fails."""
    ob = np.asarray(off_b, np.float64).reshape(NH, NP, 2)
    info = []
    for h in range(NH):
        u = None
        for p in range(NP):
            fx, fy = ob[h, p, 0], ob[h, p, 1]
            dx, dy = round(fx), round(fy)
            # must be integer shifts with negligible bilinear remainder
            if abs(fx - dx) > 1e-6 or abs(fy - dy) > 1e-6:
                return None
            if dx % (p + 1) or dy % (p + 1):
                return None
            uu = (dx // (p + 1), dy // (p + 1))
            if abs(uu[0]) > 1 or abs(uu[1]) > 1:
                return None
            if u is None:
                u = uu
            elif u != uu:
                return None
        info.append(u)
    plus = [h for h in range(NH) if info[h][0] == 1]
    minus = [h for h in range(NH) if info[h][0] == -1]
    zero = [h for h in range(NH) if info[h][0] == 0]
    if len(plus) != 3 or len(minus) != 3 or len(zero) != 2:
        return None
    hord = plus + minus + zero
    uys = [info[h][1] for h in hord]
    uxs = [info[h][0] for h in hord]
    return hord, uys, uxs


def _build_smats():
    """smat[k] for k in -4..4: m[j_in, j_out] = 1 at j_in = j_out + k."""
    mats = np.zeros((9, P, P), np.float32)
    for k in range(-4, 5):
        m = mats[k + 4]
        for j_out in range(W):
            j_in = j_out + k
            if 0 <= j_in < W:
                m[j_in, j_out] = 1.0
    return mats


def _np_reference(x, v_w, v_b, aw_w, aw_b, off_w, off_b, proj_w, proj_b, Hh, Ww):
    """Pure-numpy fallback mirroring reference.py (only used off-spec)."""
    B, N, C = x.shape
    v = (x @ v_w.T + v_b).reshape(B, N, NH, HD).transpose(0, 2, 1, 3)
    v = v.reshape(B * NH, Hh, Ww, HD)
    mh, mw = np.meshgrid(np.arange(Hh, dtype=x.dtype), np.arange(Ww, dtype=x.dtype),
                         indexing="ij")
    ref = np.stack([mw, mh], -1).reshape(1, N, 1, 2)
    off = (x @ off_w.T + off_b).reshape(B, N, NH, NP, 2).transpose(0, 2, 1, 3, 4)
    off = off.reshape(B * NH, N, NP, 2)
    grid = ref + off
    w = (x @ aw_w.T + aw_b).reshape(B, N, NH, NP).transpose(0, 2, 1, 3)
    w = w.reshape(B * NH, N, NP)
    w = np.exp(w - w.max(-1, keepdims=True))
    w = w / w.sum(-1, keepdims=True)
    G = B * NH
    vf = v.reshape(G, Hh * Ww, HD)
    gx, gy = grid[..., 0], grid[..., 1]
    x0 = np.floor(gx); y0 = np.floor(gy)
    wx1 = gx - x0; wx0 = 1.0 - wx1
    wy1 = gy - y0; wy0 = 1.0 - wy1
    x0i = x0.astype(np.int64); y0i = y0.astype(np.int64)

    def gather(xi, yi):
        valid = (xi >= 0) & (xi < Ww) & (yi >= 0) & (yi < Hh)
        idx = (np.clip(yi, 0, Hh - 1) * Ww + np.clip(xi, 0, Ww - 1))
        g = np.take_along_axis(vf, idx.reshape(G, -1, 1), axis=1)
        return g.reshape(*xi.shape, HD) * valid[..., None]

    samp = ((wy0 * wx0)[..., None] * gather(x0i, y0i)
            + (wy0 * wx1)[..., None] * gather(x0i + 1, y0i)
            + (wy1 * wx0)[..., None] * gather(x0i, y0i + 1)
            + (wy1 * wx1)[..., None] * gather(x0i + 1, y0i + 1))
    out = np.einsum("gnpd,gnp->gnd", samp, w)
    out = out.reshape(B, NH, N, HD).transpose(0, 2, 1, 3).reshape(B, N, C)
    return (out @ proj_w.T + proj_b).astype(np.float32)


# ---------------------------------------------------------------------------
# device program
# ---------------------------------------------------------------------------

def _build_program(uys):
    import concourse.bass as bass
    import concourse.mybir as mybir
    import concourse.tile as tile
    from concourse import bacc

    dt = mybir.dt
    f32 = dt.float32
    bf16 = dt.bfloat16

    # engine placement knobs. PSUM evacuations may only use scalar/vector
    # (GPSIMD cannot access PSUM on HW); pool takes SBUF-only weighting ops.
    EV_A = os.environ.get("EV_A", "sv")        # rotation for A evacs
    EV_OUT = os.environ.get("EV_OUT", "s")     # rotation for out evacs
    EV_OT0 = os.environ.get("EV_OT0", "s")     # D0 ot evacs
    EV_OT1 = os.environ.get("EV_OT1", "v")     # D1 ot evacs
    EV_Y0 = os.environ.get("EV_Y0", "sv")      # D0 y evacs (mc0, mc1)
    EV_Y1 = os.environ.get("EV_Y1", "ss")      # D1 y evacs
    WEIGHT_ROT = os.environ.get("WEIGHT_ROT", "vvvp")  # weighting ops rotation
    ENG = {"s": "scalar", "v": "vector", "p": "pool"}

    nc = bacc.Bacc("TRN2", target_bir_lowering=False, debug=False,
                   num_devices=N_CORES)

    _rotc = {}

    def evac(which):
        if len(which) > 1:  # rotation string like "sv"
            i = _rotc[which] = (_rotc.get(which, -1) + 1) % len(which)
            which = which[i]
        return {"s": nc.scalar.copy, "v": nc.vector.tensor_copy,
                "scalar": nc.scalar.copy, "vector": nc.vector.tensor_copy,
                "pool": nc.gpsimd.tensor_copy}[which]

    def tt_eng(rot):
        i = _rotc[rot] = (_rotc.get(rot, -1) + 1) % len(rot)
        return {"v": nc.vector, "p": nc.gpsimd, "s": None}[rot[i]]

    INTERLEAVE_C = os.environ.get("INTERLEAVE_C", "0") == "1"

    # ---- DRAM I/O ----
    xt_d = nc.dram_tensor("xt_dev", [DIM, TOK_V], bf16, kind="ExternalInput")
    wb_d = nc.dram_tensor("wb_cat", [2, P, NCH], bf16, kind="ExternalInput")
    s_d = nc.dram_tensor("s_mats", [9, P, P], bf16, kind="ExternalInput")
    pj_d = nc.dram_tensor("proj_t", [2, 2, P, P], bf16, kind="ExternalInput")
    id_d = nc.dram_tensor("ident", [P, P], bf16, kind="ExternalInput")
    y_d = [nc.dram_tensor(f"y{mc}", [P, ROWS_OUT * W], bf16,
                          kind="ExternalOutput") for mc in range(2)]

    NG = 9  # x DMA groups of 8 rows

    with tile.TileContext(nc) as tc:
        with (
            tc.tile_pool(name="const", bufs=1) as cpool,
            tc.tile_pool(name="big", bufs=1) as bigpool,
            tc.tile_pool(name="stA", bufs=2) as stA,
            tc.tile_pool(name="stB", bufs=2) as stB,
            tc.tile_pool(name="stM", bufs=1) as stM,
            tc.tile_pool(name="stD", bufs=2) as stD,
            tc.tile_pool(name="psA", bufs=2, space="PSUM") as psA,
            tc.tile_pool(name="psC", bufs=2, space="PSUM") as psC,
        ):
            # ---- constants (only wb gates phase A; rest loaded later).
            # wb is split per contraction half so the first matmul only
            # waits for its own half's DMA. ----
            wb_sb = cpool.tile([P, 2, NCH], bf16, tag="wb")
            wb_kc = [wb_sb[:, 0, :], wb_sb[:, 1, :]]

            def load_wb():
                nc.sync.dma_start(wb_sb[:],
                                  wb_d.rearrange("kc k f -> k kc f"))
            s_sb = cpool.tile([P, 9, P], bf16, tag="smats")
            pj_sb = cpool.tile([P, 2, 2, P], bf16, tag="proj")
            id_sb = cpool.tile([P, P], bf16, tag="ident")

            def load_late_consts():
                nc.sync.dma_start(s_sb[:], s_d.rearrange("s k f -> k s f"))
                nc.sync.dma_start(pj_sb[:],
                                  pj_d.rearrange("kc m k f -> k kc m f"))
                nc.sync.dma_start(id_sb[:], id_d[:])

            def sf(dx):  # forward shift matrix index
                return dx + 4

            def sb(dx):  # backward (S_-dx) index
                return -dx + 4

            # ---- persistent tiles ----
            # VL: [j, 288 ch, 72 row-slots]; ch = h'*32+d for v, 256+h'*4+p logits
            vl = bigpool.tile([P, NCH, ROWS_V], bf16, tag="VL")
            es = [bigpool.tile([P, NH * NP, 32], bf16, tag="E", name=f"e{h}")
                  for h in range(2)]
            eps = [bigpool.tile([P, 8, 3, 32], bf16, tag="EP", name=f"ep{h}")
                   for h in range(2)]
            outs = [bigpool.tile([P, DIM, 32], bf16, tag="OUT", name=f"out{h}")
                    for h in range(2)]

            def phase_a_dma(row0, nrows, mid=None):
                xt_g = [stA.tile([P, 2048], bf16, tag=f"xt{kc}", bufs=3,
                                 name=f"xtg{kc}") for kc in range(2)]
                for kc in range(2):
                    nc.sync.dma_start(
                        xt_g[kc][:, :P * nrows],
                        xt_d[P * kc:P * kc + P,
                             P * row0:P * (row0 + nrows)])
                    if kc == 0 and mid is not None:
                        mid()
                return xt_g

            def phase_a(row0, nrows, early, after_dma=None, xt_g=None):
                """x rows row0..row0+nrows: fused v+logit projection.

                Single-row PSUM tiles with a 4-deep ring: the evac round-trip
                (copy + 2 sem hops ~= 850ns) hides under 4 rows of PE work."""
                if xt_g is None:
                    xt_g = phase_a_dma(row0, nrows, mid=after_dma)
                for rl in range(nrows):
                    a_ps = psA.tile([P, 512], f32, tag="a", bufs=4)
                    for kc in range(2):
                        nc.tensor.matmul(
                            a_ps[:, :NCH],
                            xt_g[kc][:, P * rl:P * rl + P],
                            wb_kc[kc][:], start=(kc == 0),
                            stop=(kc == 1))
                    s0 = row0 + rl
                    evac(EV_A)(vl[:, :, s0], a_ps[:, :NCH])

            def phase_b(half, eng=None):
                """exp + softmax over the 4 points, all heads, 32 rows."""
                eng = eng or nc.vector
                rr = 32 * half
                e_sb = es[half]
                nc.scalar.activation(
                    e_sb[:], vl[:, DIM:, HALO + rr:HALO + rr + 32],
                    mybir.ActivationFunctionType.Exp)
                ev = e_sb[:].rearrange("j (h p) i -> j h p i", p=NP)
                z0 = stB.tile([P, NH, 32], f32, tag="z0")
                z1 = stB.tile([P, NH, 32], f32, tag="z1")
                zr = stB.tile([P, NH, 32], f32, tag="zr")
                zb = stB.tile([P, NH, 32], bf16, tag="zb")
                eng.tensor_tensor(z0[:], ev[:, :, 0, :], ev[:, :, 1, :],
                                  op=mybir.AluOpType.add)
                eng.tensor_tensor(z1[:], ev[:, :, 2, :], ev[:, :, 3, :],
                                  op=mybir.AluOpType.add)
                eng.tensor_tensor(z0[:], z0[:], z1[:],
                                  op=mybir.AluOpType.add)
                nc.vector.reciprocal(zr[:], z0[:])
                nc.vector.tensor_copy(zb[:], zr[:])
                eng.tensor_tensor(
                    ev[:], ev[:],
                    zb[:].unsqueeze(2).broadcast_to([P, NH, NP, 32]),
                    op=mybir.AluOpType.mult)

            def phase_ep(half):
                """column-shifted weights E' = S_-dx^T E, batched 3 heads/dx."""
                e_sb = es[half]
                for rnd in range(2):           # rnd 0: dx=+1..+4, 1: dx=-1..-4
                    sgn = 1 if rnd == 0 else -1
                    hoff = 0 if rnd == 0 else 3
                    ep_ps = psC.tile([P, 4, 128], f32, tag="ep", bufs=1)
                    for k in range(1, 5):
                        dx = sgn * k
                        # heads hoff..hoff+2, point k-1: slots h'*4+(k-1)
                        rhs = e_sb[:].rearrange("j (h p) i -> j h p i", p=NP)[
                            :, hoff:hoff + 3, k - 1, :]
                        nc.tensor.matmul(ep_ps[:, k - 1, :96],
                                         s_sb[:, sb(dx), :], rhs,
                                         start=True, stop=True)
                    evac(os.environ.get("EV_EP", "s"))(
                        eps[half][:, 4 * rnd:4 * rnd + 4],
                        ep_ps[:, :, :96].rearrange("j k (t i) -> j k t i", t=3))

            def _weight_slice(half, hp, p):
                """weight row [P, 32] for (permuted head hp, point p)."""
                if hp < 6:
                    rnd = 0 if hp < 3 else 1
                    return eps[half][:, 4 * rnd + p, hp % 3, :]
                return es[half][:, NP * hp + p, :]

            WEIGHT_BATCH = os.environ.get("WEIGHT_BATCH", "0") == "1"

            def _c_weight(half, m_t, t, hp):
                """weighting multiplies for one head into m_t[:, t]."""
                rr = 32 * half
                uy = uys[hp]
                if WEIGHT_BATCH:
                    # one op per head: overlapping-window AP over p
                    s1 = rr + HALO + uy
                    v_ap = vl[:, HD * hp:HD * hp + HD, s1:s1 + 32] \
                        .unsqueeze(1).broadcast_to([P, NP, HD, 32])
                    v_ap.ap[1] = [uy, NP]
                    w_ap = _weight_slice(half, hp, 0) \
                        .unsqueeze(1).unsqueeze(1) \
                        .broadcast_to([P, NP, HD, 32])
                    w_ap.ap[1] = [96, NP] if hp < 6 else [32, NP]
                    tt_eng(WEIGHT_ROT).tensor_tensor(
                        m_t[:, t], v_ap, w_ap, op=mybir.AluOpType.mult)
                    return
                for p in range(NP):
                    s0 = rr + HALO + uy * (p + 1)
                    tt_eng(WEIGHT_ROT).tensor_tensor(
                        m_t[:, t, p],
                        vl[:, HD * hp:HD * hp + HD, s0:s0 + 32],
                        _weight_slice(half, hp, p)
                        .unsqueeze(1).broadcast_to([P, HD, 32]),
                        op=mybir.AluOpType.mult)

            def _c_shift(half, m_t, t, hp, out_eng):
                """shift-accumulate + out evac for one head."""
                dxu = 1 if hp < 3 else (-1 if hp < 6 else 0)
                o_ps = [psC.tile([P, 512], f32, tag="o", bufs=3,
                                 name=f"o{ch}") for ch in range(2)]
                for p in range(NP):
                    dx = dxu * (p + 1)
                    mv = m_t[:, t, p].rearrange("j d i -> j (d i)")
                    for ch in range(2):
                        nc.tensor.matmul(
                            o_ps[ch][:], s_sb[:, sf(dx), :],
                            mv[:, 512 * ch:512 * ch + 512],
                            start=(p == 0), stop=(p == NP - 1))
                for ch in range(2):
                    evac(out_eng or EV_OUT)(
                        outs[half][:, HD * hp + 16 * ch:HD * hp + 16 * ch + 16, :],
                        o_ps[ch][:].rearrange("j (d i) -> j d i", d=16))

            def phase_c_heads(half, heads, mtag, out_eng=None):
                """weighting + shift-accumulate + evac for a group of heads."""
                m_t = stM.tile([P, 3, NP, 32, 32], bf16,
                               tag=mtag, bufs=1, name=mtag)
                if INTERLEAVE_C:
                    for t, hp in enumerate(heads):
                        _c_weight(half, m_t, t, hp)
                        _c_shift(half, m_t, t, hp, out_eng)
                else:
                    for t, hp in enumerate(heads):
                        _c_weight(half, m_t, t, hp)
                    for t, hp in enumerate(heads):
                        _c_shift(half, m_t, t, hp, out_eng)

            # ---- phase D: software-pipelined (trans(g+1) emitted before
            # proj(g) so PE never stalls on the ot evacuation copy) ----
            d_state = {}

            def d_trans(half, gl, ot_eng=None):
                ot_eng = ot_eng or (EV_OT0 if half == 0 else EV_OT1)
                """transposes of out-row group gl to channel-major."""
                i0 = 4 * gl
                ot_ps = psA.tile([P, 8, P], bf16, tag="a",
                                 name="otp", bufs=4)
                for kc in range(2):
                    for il in range(4):
                        nc.tensor.transpose(
                            ot_ps[:, 4 * kc + il, :],
                            outs[half][:, P * kc:P * kc + P, i0 + il],
                            id_sb[:])
                ot_sb = stD.tile([P, 2, 512], bf16, tag="ot", bufs=2)
                evac(ot_eng)(
                    ot_sb[:].rearrange("j kc f -> j (kc f)"),
                    ot_ps[:].rearrange("j a b -> j (a b)"))
                d_state[(half, gl)] = ot_sb

            def d_proj(half, gl, y_engs=None):
                """output projection + y evac (+DMA every 4th group)."""
                y_engs = y_engs or (EV_Y0 if half == 0 else EV_Y1)
                if half == 1 and gl >= 6:
                    y_engs = "sv"   # parallel evac on the critical tail
                g = 8 * half + gl
                ot_sb = d_state.pop((half, gl))
                y_ps = [psA.tile([P, 512], f32, tag="a",
                                 name=f"yps{mc}", bufs=4) for mc in range(2)]
                for mc in range(2):
                    for kc in range(2):
                        nc.tensor.matmul(
                            y_ps[mc][:], pj_sb[:, kc, mc, :],
                            ot_sb[:, kc, :],
                            start=(kc == 0), stop=(kc == 1))
                q = gl % 2
                ysb = d_state.get(("ysb", half, gl // 2))
                if ysb is None:
                    ysb = stD.tile([P, 2, 2, 512], bf16, tag="y", bufs=3)
                    d_state[("ysb", half, gl // 2)] = ysb
                for mc in range(2):
                    evac(y_engs[mc])(ysb[:, mc, q, :], y_ps[mc][:])
                if q == 1:
                    g0 = 512 * (g - 1)
                    for mc in range(2):
                        nc.sync.dma_start(
                            y_d[mc][:, g0:g0 + 1024], ysb[:, mc, :, :])

            # ---- emission ----
            phase_a(0, 8, True, after_dma=load_wb)
            phase_a(8, 8, True)
            phase_a(16, 16, True)
            load_late_consts()
            phase_a(32, 16, True)
            phase_b(0)           # no PE work; overlaps A tail
            phase_a(48, 16, False)
            phase_ep(0)
            phase_c_heads(0, [0, 1, 2], "m0")
            phase_a(64, 4, False)
            phase_c_heads(0, [3, 4, 5], "m1")
            xt68 = phase_a_dma(68, 4)   # prefetch halo rows
            phase_c_heads(0, [6, 7], "m0", "sv")
            phase_a(68, 4, False, xt_g=xt68)  # fills the duo->B1 gap
            phase_b(1)
            d_trans(0, 0)   # fill the B1 latency with D0 groups
            d_trans(0, 1); d_proj(0, 0)
            d_trans(0, 2); d_proj(0, 1)
            d_trans(0, 3); d_proj(0, 2)
            phase_ep(1)
            d_trans(0, 4); d_proj(0, 3)
            phase_c_heads(1, [0, 1, 2], "m1")
            d_trans(0, 5); d_proj(0, 4)
            phase_c_heads(1, [3, 4, 5], "m0")
            d_trans(0, 6); d_proj(0, 5)
            phase_c_heads(1, [6, 7], "m1", "sv")
            d_trans(0, 7); d_proj(0, 6)
            d_proj(0, 7)
            d_trans(1, 0)
            for gl in range(1, 8):
                d_trans(1, gl); d_proj(1, gl - 1)
            d_proj(1, 7)

    nc.compile()
    return nc


# ---------------------------------------------------------------------------
# host wrapper
# ---------------------------------------------------------------------------

def kernel(x, v_w, v_b, aw_w, aw_b, off_w, off_b, proj_w, proj_b, H=128, W=128,
           **_unused):
    import ml_dtypes
    bf = ml_dtypes.bfloat16

    x = np.ascontiguousarray(np.asarray(x, np.float32))
    v_w = np.asarray(v_w, np.float32); v_b = np.asarray(v_b, np.float32)
    aw_w = np.asarray(aw_w, np.float32); aw_b = np.asarray(aw_b, np.float32)
    off_w = np.asarray(off_w, np.float32); off_b = np.asarray(off_b, np.float32)
    proj_w = np.asarray(proj_w, np.float32); proj_b = np.asarray(proj_b, np.float32)

    geom = _derive_geometry(off_b)
    if (np.any(off_w != 0.0) or int(H) != 128 or int(W) != 128 or geom is None
            or np.any(v_b) or np.any(aw_b) or np.any(proj_b)):
        return _np_reference(x, v_w, v_b, aw_w, aw_b, off_w, off_b,
                             proj_w, proj_b, int(H), int(W))
    hord, uys, _uxs = geom

    key = ("prog2", tuple(uys))
    if key not in _cache:
        _cache[key] = _build_program(uys)
    nc = _cache[key]

    B = x.shape[0]
    # ---- host prep (shared across cores) ----
    # channel permutation: ch' = h'*32+d  <-  orig h*32+d
    vperm = np.concatenate([np.arange(HD) + HD * h for h in hord])
    aperm = np.concatenate([np.arange(NP) + NP * h for h in hord])
    wb_cat = np.empty((2, P, NCH), np.float32)
    v_wp = v_w[vperm]          # [256 out-ch', 256 in]
    aw_wp = aw_w[aperm]        # [32 out-ch', 256 in]
    for kc in range(2):
        wb_cat[kc, :, :DIM] = v_wp[:, P * kc:P * (kc + 1)].T
        wb_cat[kc, :, DIM:] = aw_wp[:, P * kc:P * (kc + 1)].T
    pj_perm = proj_w[:, vperm]  # permute contraction columns
    pj_t = np.empty((2, 2, P, P), np.float32)
    for kc in range(2):
        for mc in range(2):
            pj_t[kc, mc] = pj_perm[P * mc:P * (mc + 1), P * kc:P * (kc + 1)].T
    shared = dict(wb_cat=np.ascontiguousarray(wb_cat).astype(bf),
                  s_mats=np.ascontiguousarray(_build_smats()).astype(bf),
                  proj_t=np.ascontiguousarray(pj_t).astype(bf),
                  ident=np.eye(P, dtype=np.float32).astype(bf))

    xr = x.reshape(B, H, W, DIM)
    in_maps = []
    for d in range(N_CORES):
        b, half = d // 2, d % 2
        r0 = ROWS_OUT * half
        x_dev = np.zeros((ROWS_V, W, DIM), np.float32)
        lo, hi = max(0, r0 - HALO), min(H, r0 + ROWS_OUT + HALO)
        x_dev[lo - (r0 - HALO):hi - (r0 - HALO)] = xr[b, lo:hi]
        m = dict(shared)
        m["xt_dev"] = np.ascontiguousarray(
            x_dev.reshape(TOK_V, DIM).T).astype(bf)
        in_maps.append(m)

    from concourse import bass_utils
    res = bass_utils.run_bass_kernel_spmd(
        nc, in_maps, core_ids=list(range(N_CORES)),
        trace=os.environ.get("KERNEL_TRACE", "0") == "1")
    kernel.last_results = res

    y = np.empty((B, N_TOK, DIM), np.float32)
    for d in range(N_CORES):
        b, half = d // 2, d % 2
        yd = np.concatenate([np.asarray(res.results[d]["y0"]),
                             np.asarray(res.results[d]["y1"])], 0)
        y[b, ROWS_OUT * W * half:ROWS_OUT * W * (half + 1), :] = \
            yd.astype(np.float32).T
    return y
